# revision 8
# baseline (speedup 1.0000x reference)
"""CGCNNConv forward on 8 Trainium2 NeuronCores (Bass/Tile).

Math (per atom i, neighbor slot m):
  combined = [atom[i] | atom[nbr[i,m]] | bond[i,m]]          # 640
  z        = combined @ fc_w.T + fc_b                        # 512
  z        = LN(z) * ln1_g + ln1_b
  out[i]   = atom[i] + LN( mean_m sigmoid(z[:256]) * softplus(z[256:]) ) * ln2_g + ln2_b

Sharding: atoms split across 8 cores (padded 30000 -> 30720 = 8*3840).
atom_feats is replicated to every core's HBM (bf16) so the neighbor
gather is a local dma_gather.

Device layout per core (supertile = 128 atoms = 1536 (atom,m) rows = 12
row-tiles of 128 rows):
  - atom contribution computed once per atom (atomT stationary, W1^T
    moving) with fc_b folded in, then expanded to rows with a one-hot
    matmul (E).
  - neighbor rows arrive via TWO 768-row dma_gather ops per supertile
    (row-major bf16; transpose-mode gather measured ~2x slower on HW,
    and >768 rows per gather crashes the SWDGE ucode).  Two gather
    instructions replace twelve per-row-tile indirect DMAs, cutting the
    994ns SWDGE fixed cost per op; the DMA stream itself is descriptor-
    rate-bound at ~11.5ns/row.
  - gathered rows are PE-transposed (bf16, 2x[128,128] per row-tile)
    and copied PSUM->SBUF on ACT (Copy lives in every act table set).
  - bond features arrive pre-transposed from the host.
  - z accumulates in PSUM [128 rows, 512]; LN1 stats via bn_stats; the
    rsqrt(var) chain runs on ACT batched over STAT_BB row-tiles
    ([128,BB] Ln+Exp instead of per-tile [128,1] ops).
  - sigmoid/softplus use exp/ln only, all from the combined
    natural_log_exp activation-table set pre-loaded once up front --
    otherwise the table-load pass flip-flops between the exp-only and
    ln-only sets (2 x ~1.3us reload per row-tile, ~1ms total).
  - all elementwise work stays OFF gpsimd: mixing Pool tensor ops with
    Pool DMA ops forces a ~6us ucode library reload per switch
    (HW-measured +1.5ms).
  - mean over m via a one-hot 1/12 matmul (G) accumulating [128 atoms,256]
    across the 12 row-tiles of a supertile; LN2 + residual epilogue.
Matmuls run in float32r (full-rate at N>=256) except the neighbor
contribution (bf16 gather data, bf16 weight copy).
"""

import os
import sys

import numpy as np

sys.path.insert(0, "/opt/trn_rl_repo")
os.environ.setdefault("NEURON_COMPILE_CACHE_URL", "/root/neff_cache")

N, M, A, B = 30000, 12, 256, 128
NCORES = 8
NS = 3840                    # atoms per core (padded)
NPAD = NS * NCORES           # 30720
SA = 128                     # atoms per supertile
NSUPER = NS // SA            # 30
RT = 12                      # row-tiles per supertile
ST_ROWS = SA * M             # 1536
ROWS = NS * M                # 46080
K2A = 2 * A                  # 512
KIN = 2 * A + B              # 640
NCHUNK = KIN // 128          # 5
LN_EPS = 1e-5
STAT_BB = 3                  # row-tiles per batched LN1-rsqrt group

_CACHE = {}
_NEFF_CACHE_DIR = os.environ.get("NEFF_DISK_CACHE", "/root/neff_cache")
_cache_installed = False

_SPSIG_CONSTS = (1.0, -0.23549792, 2.0017324)


def _register_sp_sigmoid_op():
    """Custom DVE op: out = Src1 * approx(1/(Src0 + 1)).

    Fuses the sigmoid-gate divide chain {denom = 1+e_u; rden = 1/denom;
    gated = sp*rden} (3 DVE instructions, ~760ns/row-tile) into ONE
    custom-DVE instruction.  Same BITWISE_NOT exponent-flip seed as
    RECIPROCAL_APPROX_FAST but on (Src0+1), with a single inline NR pass
    (the +1 stage uses the uop slot the second NR pass would need):
    ~0.17% max rel err on the gate, well inside the 2e-2 gate."""
    import numpy as np

    from concourse import dve_ops
    from concourse.dve_spec import AluOp, Bin, C0, C1, C2, Spec, Src0, Src1
    from concourse.dve_spec import _has_src1, lower
    from concourse.dve_uop import DveOpSpec

    name = "SP_SIGMOID_ANT"
    if name in dve_ops._SUB_OPCODE_FOR_NAME:
        return next(op for op in dve_ops.OPS if op.name == name)

    s = Src0 + C0
    not_s = Bin(AluOp.BITWISE_NOT, s, s)
    y0 = not_s * C1
    y1 = y0 * (C2 - s * y0)
    body = y1 * Src1

    def ref(in0, in1, s0, s1, imm2):
        sv = (in0.astype(np.float32) + np.float32(s0)).astype(np.float32)
        nx = (~np.ascontiguousarray(sv).view(np.int32)).view(np.float32)
        yy0 = nx * np.float32(s1)
        yy1 = yy0 * (np.float32(imm2) - sv * yy0)
        return (yy1 * in1).astype(np.float32)

    spec = Spec(body=body, reference=ref)
    opcode = max(dve_ops._SUB_OPCODE_FOR_NAME.values()) + 1
    assert opcode < 0x20
    shas = {}
    for ver in ("v3", "v4"):
        op_spec = DveOpSpec(name=name, opcode=opcode, uops=lower(spec, ver=ver),
                            rd1_en=_has_src1(spec))
        shas[ver] = op_spec.sha(ver)
    op = dve_ops.DveOp(name, spec, subdim=False, uops_sha=shas)
    dve_ops.OPS.append(op)
    dve_ops._SUB_OPCODE_FOR_NAME[name] = opcode
    dve_ops.CUSTOM_DVE_SPECS[name] = spec
    return op


def _install_neff_cache():
    """Cache compiled NEFFs on disk keyed by BIR hash."""
    global _cache_installed
    if _cache_installed:
        return
    _cache_installed = True
    import hashlib
    import shutil

    from concourse import bass2jax, bass_utils

    orig = bass_utils.compile_bir_kernel

    def cached(bir_json, tmpdir, neff_name="file.neff"):
        try:
            os.makedirs(_NEFF_CACHE_DIR, exist_ok=True)
            h = hashlib.sha256(bir_json).hexdigest()[:32]
            cpath = os.path.join(_NEFF_CACHE_DIR, h + ".neff")
            if os.path.exists(cpath):
                dst = os.path.join(tmpdir, neff_name)
                shutil.copy(cpath, dst)
                return dst
        except Exception:
            cpath = None
        out = orig(bir_json, tmpdir, neff_name)
        if cpath is not None:
            try:
                shutil.copy(out, cpath)
            except Exception:
                pass
        return out

    bass_utils.compile_bir_kernel = cached
    bass2jax.compile_bir_kernel = cached


def _expand_maps():
    """E: [atom a, (j,r)] one-hot; G: [row r, (j,a)] one-hot / 12."""
    emat = np.zeros((SA, RT * 128), dtype=np.float32)
    gmat = np.zeros((128, RT * 128), dtype=np.float32)
    for j in range(RT):
        for r in range(128):
            a = (128 * j + r) // M
            emat[a, j * 128 + r] = 1.0
            gmat[r, j * 128 + a] = 1.0 / M
    return emat, gmat


def _build(general_ln1, general_ln2):
    import concourse.bass as bass
    import concourse.tile as tile
    from concourse import bacc, mybir
    from concourse.hw_specs import get_activation_tables

    f32 = mybir.dt.float32
    f32r = mybir.dt.float32r
    bf16 = mybir.dt.bfloat16
    i16 = mybir.dt.int16
    AF = mybir.ActivationFunctionType

    global _SPSIG_OP
    _SPSIG_OP = _register_sp_sigmoid_op()

    nc = bacc.Bacc("TRN2", target_bir_lowering=False, debug=False,
                   num_devices=NCORES, num_swdge_queues=4)

    d_atom_bf16 = nc.dram_tensor("atom_bf16", [NPAD, A], bf16,
                                 kind="ExternalInput")
    d_atomT = nc.dram_tensor("atomT2", [128, 2 * NS], f32r, kind="ExternalInput")
    d_atom_rows = nc.dram_tensor("atom_rows", [NS, A], f32, kind="ExternalInput")
    d_bondT = nc.dram_tensor("bondT", [B, ROWS], bf16, kind="ExternalInput")
    d_idx16 = nc.dram_tensor("idx16", [128, ROWS // 16], i16,
                             kind="ExternalInput")
    d_wt = nc.dram_tensor("wt", [128, NCHUNK * K2A], f32r, kind="ExternalInput")
    d_wtn = nc.dram_tensor("wtn_bf16", [128, 3 * K2A], bf16,
                           kind="ExternalInput")
    d_fcb = nc.dram_tensor("fcb_rep", [128, K2A], f32, kind="ExternalInput")
    d_emat = nc.dram_tensor("emat", [SA, RT * 128], f32r, kind="ExternalInput")
    d_gmat = nc.dram_tensor("gmat", [128, RT * 128], f32r, kind="ExternalInput")
    d_ident = nc.dram_tensor("ident_bf16", [128, 128], bf16,
                             kind="ExternalInput")
    if general_ln1:
        d_g1 = nc.dram_tensor("ln1g_rep", [128, K2A], f32, kind="ExternalInput")
        d_b1 = nc.dram_tensor("ln1b_rep", [128, K2A], f32, kind="ExternalInput")
    if general_ln2:
        d_g2 = nc.dram_tensor("ln2g_rep", [128, A], f32, kind="ExternalInput")
        d_b2 = nc.dram_tensor("ln2b_rep", [128, A], f32, kind="ExternalInput")
    d_out = nc.dram_tensor("out", [NS, A], f32, kind="ExternalOutput")

    r = lambda ap: ap if ap.dtype == f32r else ap.bitcast(f32r)

    # Index of the combined ln+exp activation table set.  The table-load
    # placement pass inserts a load wherever an activation's function is
    # missing from the currently-loaded set, picking the FIRST set that
    # contains it -- Exp alone resolves to the exp-only set and Ln to the
    # ln-only set, so a kernel interleaving them would reload tables twice
    # per row-tile (~1.3us each).  Pre-loading the combined set (which has
    # both) up front satisfies every activation and the pass adds nothing.
    lnexp_set_id = list(get_activation_tables(nc.m.arch).keys()).index(
        "natural_log_exp_and_others")

    with tile.TileContext(nc) as tc:
        nc.scalar.add_instruction(mybir.InstLoadActFuncSet(
            name=nc.get_next_instruction_name(), ins=[], outs=[],
            act_func_set_id=lnexp_set_id))
        with (
            tc.tile_pool(name="const", bufs=1) as cpool,
            tc.tile_pool(name="io", bufs=3) as iopool,
            tc.tile_pool(name="work", bufs=4) as wpool,
            tc.tile_pool(name="stat", bufs=4) as spool,
            tc.tile_pool(name="gat", bufs=2 * RT) as gatpool,
            tc.tile_pool(name="zps", bufs=STAT_BB, space="PSUM") as zpool,
            tc.tile_pool(name="tps", bufs=2, space="PSUM") as tpool,
            tc.tile_pool(name="aps", bufs=1, space="PSUM") as apool,
            tc.tile_pool(name="gps", bufs=2, space="PSUM") as gpool,
        ):
            # ---- resident constants ----
            wt = cpool.tile([128, NCHUNK * K2A], f32r, tag="wt")
            nc.sync.dma_start(wt[:], d_wt[:])
            wtn = cpool.tile([128, 3 * K2A], bf16, tag="wtn")
            nc.sync.dma_start(wtn[:], d_wtn[:])
            atomT = cpool.tile([128, 2 * NS], f32r, tag="atomT")
            nc.sync.dma_start(atomT[:], d_atomT[:])
            fcb = cpool.tile([128, K2A], f32, tag="fcb")
            nc.sync.dma_start(fcb[:], d_fcb[:])
            emat = cpool.tile([SA, RT * 128], f32r, tag="emat")
            nc.sync.dma_start(emat[:], d_emat[:])
            gmat = cpool.tile([128, RT * 128], f32r, tag="gmat")
            nc.sync.dma_start(gmat[:], d_gmat[:])
            ident = cpool.tile([128, 128], bf16, tag="ident")
            nc.sync.dma_start(ident[:], d_ident[:])
            eps_t = cpool.tile([128, 1], f32, tag="eps")
            nc.gpsimd.memset(eps_t[:], LN_EPS)
            ones_t = cpool.tile([128, 1], f32, tag="ones")
            nc.gpsimd.memset(ones_t[:], 1.0)
            if general_ln1:
                g1 = cpool.tile([128, K2A], f32, tag="g1")
                nc.sync.dma_start(g1[:], d_g1[:])
                b1 = cpool.tile([128, K2A], f32, tag="b1")
                nc.sync.dma_start(b1[:], d_b1[:])
            if general_ln2:
                g2 = cpool.tile([128, A], f32, tag="g2")
                nc.sync.dma_start(g2[:], d_g2[:])
                b2 = cpool.tile([128, A], f32, tag="b2")
                nc.sync.dma_start(b2[:], d_b2[:])

            NB = RT // STAT_BB       # stat batches per supertile

            def emit_epilogue(pend):
                """Deferred per-supertile tail: the 12 G (mean-over-m)
                matmuls + LN2 + residual + output DMA.  Emitted one
                supertile late so the PE never stalls waiting for the
                current supertile's elementwise chain to produce gated."""
                ps, pagg, parows = pend
                st6b = spool.tile([128, 6], f32, tag="st6b")
                nc.vector.bn_stats(st6b[:], pagg[:])
                st2b = spool.tile([128, 2], f32, tag="st2b")
                nc.vector.bn_aggr(st2b[:], st6b[:])
                lnv2 = spool.tile([128, 1], f32, tag="lnv2")
                nc.scalar.activation(lnv2[:], st2b[:, 1:2], AF.Ln,
                                     bias=eps_t[:])
                inv2 = spool.tile([128, 1], f32, tag="inv2")
                nc.scalar.activation(inv2[:], lnv2[:], AF.Exp, scale=-0.5)
                nmi2 = spool.tile([128, 1], f32, tag="nmi2")
                nc.vector.tensor_scalar(
                    out=nmi2[:], in0=st2b[:, 0:1], scalar1=inv2[:],
                    scalar2=-1.0, op0=mybir.AluOpType.mult,
                    op1=mybir.AluOpType.mult,
                )
                normed = wpool.tile([SA, A], f32, tag="normed")
                nc.vector.tensor_scalar(
                    out=normed[:], in0=pagg[:], scalar1=inv2[:],
                    scalar2=nmi2[:], op0=mybir.AluOpType.mult,
                    op1=mybir.AluOpType.add,
                )
                if general_ln2:
                    nc.vector.tensor_mul(out=normed[:], in0=normed[:],
                                         in1=g2[:])
                    nc.vector.tensor_add(out=normed[:], in0=normed[:],
                                         in1=b2[:])
                out_sb = wpool.tile([SA, A], f32, tag="out_sb")
                nc.vector.tensor_add(out=out_sb[:], in0=normed[:],
                                     in1=parows[:])
                nc.sync.dma_start(d_out[ps * SA:(ps + 1) * SA, :], out_sb[:])

            for s in range(NSUPER):
                row0 = s * ST_ROWS
                col0 = s * (ST_ROWS // 16)
                # wrapped int16 indices for this supertile: [128, 96]
                idx = iopool.tile([128, ST_ROWS // 16], i16, tag="idx")
                nc.sync.dma_start(
                    idx[:], d_idx16[:, col0:col0 + ST_ROWS // 16])
                # gathered neighbor rows, row-major bf16, two 768-row halves
                # (transpose-mode dma_gather measured 2x slower on HW; rows
                # land [i%128 partition, i//128 block, 256] and get PE-
                # transposed below).  One gather instruction per half costs
                # ~1.3us on Pool vs 12x1.4us for per-row-tile indirect DMA.
                GH = ST_ROWS // 2     # 768
                nbr_g = iopool.tile([128, M * A], bf16, tag="nbr_g")
                for h in range(2):
                    nc.gpsimd.dma_gather(
                        out_ap=nbr_g[:, h * (GH // 128) * A:
                                     (h + 1) * (GH // 128) * A]
                        .rearrange("p (g n) -> p g n", g=GH // 128),
                        in_ap=d_atom_bf16[:],
                        idxs_ap=idx[:, h * (GH // 16):(h + 1) * (GH // 16)],
                        num_idxs=GH,
                        num_idxs_reg=GH,
                        elem_size=A,
                        transpose=False,
                        queue_num=(2 * s + h) % 4,
                    )
                # bond^T slice [128 feat, 1536 rows] bf16
                bondT = iopool.tile([B, ST_ROWS], bf16, tag="bondT")
                nc.sync.dma_start(bondT[:], d_bondT[:, row0:row0 + ST_ROWS])
                # residual rows
                arows = iopool.tile([SA, A], f32, tag="arows")
                nc.sync.dma_start(arows[:], d_atom_rows[s * SA:(s + 1) * SA, :])

                # atom contribution for these 128 atoms: [128 atoms, 512]
                ap_ps = apool.tile([SA, K2A], f32, tag="ap_ps")
                for c in range(2):
                    nc.tensor.matmul(
                        out=ap_ps[:],
                        lhsT=r(atomT[:, c * NS + s * SA: c * NS + (s + 1) * SA]),
                        rhs=r(wt[:, c * K2A:(c + 1) * K2A]),
                        start=(c == 0), stop=(c == 1),
                    )
                ap_sb = wpool.tile([SA, K2A], f32r, tag="ap_sb")
                nc.vector.tensor_add(out=ap_sb[:], in0=ap_ps[:], in1=fcb[:])

                agg = gpool.tile([SA, A], f32, tag="agg")
                gated_list = []

                for bb in range(NB):
                    zs = []
                    stb = spool.tile([128, 2 * STAT_BB], f32, tag="stb")
                    for b in range(STAT_BB):
                        j = bb * STAT_BB + b
                        # transpose gathered rows -> [feat, rows] bf16
                        tp = tpool.tile([128, A], bf16, tag="tp")
                        for c in range(2):
                            nc.tensor.transpose(
                                out=tp[:, c * 128:(c + 1) * 128],
                                in_=nbr_g[:, j * A + c * 128:
                                          j * A + (c + 1) * 128],
                                identity=ident[:],
                            )
                        nbrT = wpool.tile([128, A], bf16, tag="nbrT")
                        nc.scalar.copy(nbrT[:], tp[:])

                        # z = E@atom_part + nbrT'@W2 + bondT'@W3  [128 rows, 512]
                        z = zpool.tile([128, K2A], f32, tag="z")
                        nc.tensor.matmul(
                            out=z[:],
                            lhsT=r(emat[:, j * 128:(j + 1) * 128]),
                            rhs=r(ap_sb[:]),
                            start=True, stop=False,
                        )
                        for c in range(2):
                            nc.tensor.matmul(
                                out=z[:],
                                lhsT=nbrT[:, c * 128:(c + 1) * 128],
                                rhs=wtn[:, c * K2A:(c + 1) * K2A],
                                start=False, stop=False,
                            )
                        nc.tensor.matmul(
                            out=z[:],
                            lhsT=bondT[:, j * 128:(j + 1) * 128],
                            rhs=wtn[:, 2 * K2A:3 * K2A],
                            start=False, stop=True,
                        )
                        zs.append(z)
                        # LN1 stats into batch slot b
                        st6 = spool.tile([128, 6], f32, tag="st6")
                        nc.vector.bn_stats(st6[:], z[:])
                        nc.vector.bn_aggr(stb[:, 2 * b:2 * b + 2], st6[:])

                    # batched rsqrt chain over STAT_BB row-tiles
                    mu4 = stb[:, 0:2 * STAT_BB:2]
                    var4 = stb[:, 1:2 * STAT_BB:2]
                    lnv4 = spool.tile([128, STAT_BB], f32, tag="lnv4")
                    nc.scalar.activation(lnv4[:], var4, AF.Ln, bias=eps_t[:])
                    inv4 = spool.tile([128, STAT_BB], f32, tag="inv4")
                    nc.scalar.activation(inv4[:], lnv4[:], AF.Exp, scale=-0.5)
                    ninv4 = spool.tile([128, STAT_BB], f32, tag="ninv4")
                    nc.vector.tensor_scalar(
                        out=ninv4[:], in0=inv4[:], scalar1=-1.0, scalar2=None,
                        op0=mybir.AluOpType.mult,
                    )
                    pnmi4 = spool.tile([128, STAT_BB], f32, tag="pnmi4")
                    nc.vector.tensor_mul(out=pnmi4[:], in0=mu4, in1=inv4[:])
                    nmi4 = spool.tile([128, STAT_BB], f32, tag="nmi4")
                    nc.vector.scalar_tensor_tensor(
                        out=nmi4[:], in0=mu4, scalar=-1.0, in1=inv4[:],
                        op0=mybir.AluOpType.mult, op1=mybir.AluOpType.mult,
                    )

                    for b in range(STAT_BB):
                        j = bb * STAT_BB + b
                        z = zs[b]
                        # gate*core = ln(1+e^v) / (1+e^-u), LN1 fused via
                        # per-partition scale/bias
                        e_u = wpool.tile([128, A], f32, tag="e_u")
                        e_v = wpool.tile([128, A], f32, tag="e_v")
                        if general_ln1:
                            y = wpool.tile([128, K2A], f32, tag="y")
                            nc.vector.tensor_scalar(
                                out=y[:], in0=z[:],
                                scalar1=inv4[:, b:b + 1],
                                scalar2=nmi4[:, b:b + 1],
                                op0=mybir.AluOpType.mult,
                                op1=mybir.AluOpType.add,
                            )
                            nc.vector.tensor_mul(out=y[:], in0=y[:], in1=g1[:])
                            nc.vector.tensor_add(out=y[:], in0=y[:], in1=b1[:])
                            nc.scalar.activation(e_u[:], y[:, :A], AF.Exp,
                                                 scale=-1.0)
                            nc.scalar.activation(e_v[:], y[:, A:], AF.Exp)
                        else:
                            nc.scalar.activation(
                                e_u[:], z[:, :A], AF.Exp,
                                bias=pnmi4[:, b:b + 1], scale=ninv4[:, b:b + 1])
                            nc.scalar.activation(
                                e_v[:], z[:, A:], AF.Exp,
                                bias=nmi4[:, b:b + 1], scale=inv4[:, b:b + 1])
                        sp = wpool.tile([128, A], f32, tag="sp")
                        nc.scalar.activation(sp[:], e_v[:], AF.Ln,
                                             bias=ones_t[:])
                        # NOTE: keep elementwise OFF gpsimd -- mixing Pool
                        # tensor ops with Pool DMA ops forces a ~6us ucode
                        # library reload per switch (HW-measured +1.5ms).
                        # gated = sp/(1+e_u) in ONE custom DVE op.
                        gated = gatpool.tile([128, A], f32r, tag="gated")
                        nc.vector._custom_dve(
                            _SPSIG_OP, out=gated[:],
                            in0=e_u[:], in1=sp[:],
                            s0=_SPSIG_CONSTS[0], s1=_SPSIG_CONSTS[1],
                            imm2=_SPSIG_CONSTS[2],
                        )
                        nc.tensor.matmul(
                            out=agg[:],
                            lhsT=r(gmat[:, j * 128:(j + 1) * 128]),
                            rhs=r(gated[:]),
                            start=(j == 0), stop=(j == RT - 1),
                        )

                emit_epilogue((s, agg, arows))

    nc.compile()
    return nc


def _prep_inputs(atom_feats, bond_feats, fc_w, fc_b, ln1_g, ln1_b, ln2_g,
                 ln2_b, nbr_indices, general_ln1, general_ln2):
    import ml_dtypes

    atom_feats = np.ascontiguousarray(atom_feats, dtype=np.float32)
    pad = NPAD - N
    atom_pad = np.concatenate(
        [atom_feats, np.zeros((pad, A), np.float32)], axis=0)
    bond_pad = np.concatenate(
        [np.asarray(bond_feats, np.float32),
         np.zeros((pad, M, B), np.float32)], axis=0)
    idx_pad = np.concatenate(
        [np.asarray(nbr_indices).astype(np.int32),
         np.zeros((pad, M), np.int32)], axis=0)

    atom_bf16 = atom_pad.astype(ml_dtypes.bfloat16)

    wT = np.ascontiguousarray(np.asarray(fc_w, np.float32).T)      # [640,512]
    wt_host = np.concatenate(
        [wT[c * 128:(c + 1) * 128, :] for c in range(NCHUNK)], axis=1)
    wt_host = np.ascontiguousarray(wt_host)                         # [128,2560]
    wtn_host = np.ascontiguousarray(
        wt_host[:, 2 * K2A:5 * K2A].astype(ml_dtypes.bfloat16))     # [128,1536]
    fcb_rep = np.ascontiguousarray(
        np.broadcast_to(np.asarray(fc_b, np.float32), (128, K2A)))
    emat, gmat = _expand_maps()

    common = {"wt": wt_host, "wtn_bf16": wtn_host, "fcb_rep": fcb_rep,
              "emat": emat, "gmat": gmat, "atom_bf16": atom_bf16,
              "ident_bf16": np.eye(128, dtype=np.float32).astype(
                  ml_dtypes.bfloat16)}
    if general_ln1:
        common["ln1g_rep"] = np.ascontiguousarray(
            np.broadcast_to(np.asarray(ln1_g, np.float32), (128, K2A)))
        common["ln1b_rep"] = np.ascontiguousarray(
            np.broadcast_to(np.asarray(ln1_b, np.float32), (128, K2A)))
    if general_ln2:
        common["ln2g_rep"] = np.ascontiguousarray(
            np.broadcast_to(np.asarray(ln2_g, np.float32), (128, A)))
        common["ln2b_rep"] = np.ascontiguousarray(
            np.broadcast_to(np.asarray(ln2_b, np.float32), (128, A)))

    in_maps = []
    for i in range(NCORES):
        lo, hi = i * NS, (i + 1) * NS
        shard_atoms = atom_pad[lo:hi]                               # [3840,256]
        atomT = np.ascontiguousarray(shard_atoms.T)                 # [256,3840]
        atomT2 = np.ascontiguousarray(
            np.concatenate([atomT[:128], atomT[128:]], axis=1))     # [128,7680]
        bond_flat = bond_pad[lo:hi].reshape(ROWS, B)
        bondT = np.ascontiguousarray(
            bond_flat.T.astype(ml_dtypes.bfloat16))                 # [128,46080]
        # int16 indices, wrapped: logical index i at [i % 16, i // 16],
        # replicated down the partition dim for the 8 Q7 cores.
        flat_idx = idx_pad[lo:hi].reshape(ROWS).astype(np.int16)
        idx16 = np.ascontiguousarray(
            np.tile(flat_idx.reshape(ROWS // 16, 16).T, (8, 1)))    # [128,2880]
        m = dict(common)
        m["atomT2"] = atomT2
        m["atom_rows"] = np.ascontiguousarray(shard_atoms)
        m["bondT"] = bondT
        m["idx16"] = idx16
        in_maps.append(m)
    return in_maps


def _run(nc, in_maps, trace=False):
    from concourse.bass_utils import run_bass_kernel_spmd
    _install_neff_cache()
    res = run_bass_kernel_spmd(nc, in_maps, list(range(NCORES)), trace=trace)
    out = np.concatenate(
        [res.results[i]["out"] for i in range(NCORES)], axis=0)[:N]
    return np.ascontiguousarray(out), res


def measure_exec_ns(nc, in_maps, iters=24):
    """Estimate device exec time by pipelining async dispatches.

    No NTFF profiling is available under this axon client, so time N
    back-to-back executions of the resident executable (inputs device-
    resident, no donation) and difference out the fixed dispatch cost.
    """
    import time

    import jax
    from jax.experimental.shard_map import shard_map
    from jax.sharding import Mesh, NamedSharding, PartitionSpec

    from concourse import bass2jax, mybir
    from concourse.bass2jax import _bass_exec_p, partition_id_tensor

    bass2jax.install_neuronx_cc_hook()
    _install_neff_cache()

    partition_name = (nc.partition_id_tensor.name
                      if nc.partition_id_tensor else None)
    in_names, out_names, out_avals, zero_outs = [], [], [], []
    for alloc in nc.m.functions[0].allocations:
        if not isinstance(alloc, mybir.MemoryLocationSet):
            continue
        name = alloc.memorylocations[0].name
        if alloc.kind == "ExternalInput":
            if name != partition_name:
                in_names.append(name)
        elif alloc.kind == "ExternalOutput":
            shape = tuple(alloc.tensor_shape)
            dtype = mybir.dt.np(alloc.dtype)
            out_names.append(name)
            out_avals.append(jax.core.ShapedArray(shape, dtype))
            zero_outs.append(np.zeros(shape, dtype))
    n_params = len(in_names)
    all_in = list(in_names) + list(out_names)
    if partition_name:
        all_in.append(partition_name)

    def _body(*args):
        operands = list(args)
        if partition_name:
            operands.append(partition_id_tensor())
        outs = _bass_exec_p.bind(
            *operands, out_avals=tuple(out_avals), in_names=tuple(all_in),
            out_names=tuple(out_names), lowering_input_output_aliases=(),
            sim_require_finite=True, sim_require_nnan=True, nc=nc)
        return tuple(outs)

    devices = jax.devices()[:NCORES]
    mesh = Mesh(np.asarray(devices), ("core",))
    nin = n_params + len(zero_outs)
    sharded = jax.jit(
        shard_map(_body, mesh=mesh, in_specs=(PartitionSpec("core"),) * nin,
                  out_specs=(PartitionSpec("core"),) * len(out_names),
                  check_rep=False),
        keep_unused=True)
    sh = NamedSharding(mesh, PartitionSpec("core"))
    concat = [np.concatenate([np.asarray(in_maps[c][nm])
                              for c in range(NCORES)], axis=0)
              for nm in in_names]
    concat += [np.zeros((NCORES * z.shape[0], *z.shape[1:]), z.dtype)
               for z in zero_outs]
    dev_in = [jax.device_put(a, sh) for a in concat]

    jax.block_until_ready(sharded(*dev_in))   # compile + warm

    def run_n(n):
        t0 = time.perf_counter()
        rs = [sharded(*dev_in) for _ in range(n)]
        jax.block_until_ready(rs)
        return time.perf_counter() - t0

    run_n(2)
    t_small = min(run_n(2) for _ in range(3))
    t_big = min(run_n(2 + iters) for _ in range(3))
    est_ns = (t_big - t_small) / iters * 1e9
    return est_ns, t_small, t_big


def kernel(atom_feats, bond_feats, fc_w, fc_b, ln1_g, ln1_b, ln2_g, ln2_b,
           nbr_indices, _trace=False, _return_res=False):
    general_ln1 = not (np.allclose(ln1_g, 1.0) and np.allclose(ln1_b, 0.0))
    general_ln2 = not (np.allclose(ln2_g, 1.0) and np.allclose(ln2_b, 0.0))
    key = (general_ln1, general_ln2)
    if key not in _CACHE:
        _CACHE[key] = _build(general_ln1, general_ln2)
    nc = _CACHE[key]
    in_maps = _prep_inputs(atom_feats, bond_feats, fc_w, fc_b, ln1_g, ln1_b,
                           ln2_g, ln2_b, nbr_indices, general_ln1, general_ln2)
    out, res = _run(nc, in_maps, trace=_trace)
    if _return_res:
        return out, res
    return out



# revision 10
# speedup vs baseline: 1.0975x; 1.0975x over previous
"""CGCNNConv forward on 8 Trainium2 NeuronCores (Bass/Tile).

Math (per atom i, neighbor slot m):
  combined = [atom[i] | atom[nbr[i,m]] | bond[i,m]]          # 640
  z        = combined @ fc_w.T + fc_b                        # 512
  z        = LN(z) * ln1_g + ln1_b
  out[i]   = atom[i] + LN( mean_m sigmoid(z[:256]) * softplus(z[256:]) ) * ln2_g + ln2_b

Sharding: atoms split across 8 cores (padded 30000 -> 30720 = 8*3840).
atom_feats is replicated to every core's HBM (bf16) so the neighbor
gather is a local dma_gather.

Device layout per core (supertile = 128 atoms = 1536 (atom,m) rows = 12
row-tiles of 128 rows):
  - atom contribution computed once per atom (atomT stationary, W1^T
    moving) with fc_b folded in, then expanded to rows with a one-hot
    matmul (E).
  - neighbor rows arrive via TWO 768-row dma_gather ops per supertile
    (row-major bf16; transpose-mode gather measured ~2x slower on HW,
    and >768 rows per gather crashes the SWDGE ucode).  Two gather
    instructions replace twelve per-row-tile indirect DMAs, cutting the
    994ns SWDGE fixed cost per op; the DMA stream itself is descriptor-
    rate-bound at ~11.5ns/row.
  - gathered rows are PE-transposed (bf16, 2x[128,128] per row-tile)
    and copied PSUM->SBUF on ACT (Copy lives in every act table set).
  - bond features arrive pre-transposed from the host.
  - z accumulates in PSUM [128 rows, 512]; LN1 stats via bn_stats; the
    rsqrt(var) chain runs on ACT batched over STAT_BB row-tiles
    ([128,BB] Ln+Exp instead of per-tile [128,1] ops).
  - sigmoid/softplus use exp/ln only, all from the combined
    natural_log_exp activation-table set pre-loaded once up front --
    otherwise the table-load pass flip-flops between the exp-only and
    ln-only sets (2 x ~1.3us reload per row-tile, ~1ms total).
  - all elementwise work stays OFF gpsimd: mixing Pool tensor ops with
    Pool DMA ops forces a ~6us ucode library reload per switch
    (HW-measured +1.5ms).
  - mean over m via a one-hot 1/12 matmul (G) accumulating [128 atoms,256]
    across the 12 row-tiles of a supertile; LN2 + residual epilogue.
Matmuls run in float32r (full-rate at N>=256) except the neighbor
contribution (bf16 gather data, bf16 weight copy).
"""

import os
import sys

import numpy as np

sys.path.insert(0, "/opt/trn_rl_repo")
os.environ.setdefault("NEURON_COMPILE_CACHE_URL", "/root/neff_cache")

N, M, A, B = 30000, 12, 256, 128
NCORES = 8
NS = 3840                    # atoms per core (padded)
NPAD = NS * NCORES           # 30720
SA = 128                     # atoms per supertile
NSUPER = NS // SA            # 30
RT = 12                      # row-tiles per supertile
ST_ROWS = SA * M             # 1536
ROWS = NS * M                # 46080
K2A = 2 * A                  # 512
KIN = 2 * A + B              # 640
NCHUNK = KIN // 128          # 5
LN_EPS = 1e-5
STAT_BB = 3                  # row-tiles per batched LN1-rsqrt group

_CACHE = {}
_NEFF_CACHE_DIR = os.environ.get("NEFF_DISK_CACHE", "/root/neff_cache")
_cache_installed = False

_SPSIG_CONSTS = (1.0, -0.23549792, 2.0017324)


def _register_sp_sigmoid_op():
    """Custom DVE op: out = Src1 * approx(1/(Src0 + 1)).

    Fuses the sigmoid-gate divide chain {denom = 1+e_u; rden = 1/denom;
    gated = sp*rden} (3 DVE instructions, ~760ns/row-tile) into ONE
    custom-DVE instruction.  Same BITWISE_NOT exponent-flip seed as
    RECIPROCAL_APPROX_FAST but on (Src0+1), with a single inline NR pass
    (the +1 stage uses the uop slot the second NR pass would need):
    ~0.17% max rel err on the gate, well inside the 2e-2 gate."""
    import numpy as np

    from concourse import dve_ops
    from concourse.dve_spec import AluOp, Bin, C0, C1, C2, Spec, Src0, Src1
    from concourse.dve_spec import _has_src1, lower
    from concourse.dve_uop import DveOpSpec

    name = "SP_SIGMOID_ANT"
    if name in dve_ops._SUB_OPCODE_FOR_NAME:
        return next(op for op in dve_ops.OPS if op.name == name)

    s = Src0 + C0
    not_s = Bin(AluOp.BITWISE_NOT, s, s)
    y0 = not_s * C1
    y1 = y0 * (C2 - s * y0)
    body = y1 * Src1

    def ref(in0, in1, s0, s1, imm2):
        sv = (in0.astype(np.float32) + np.float32(s0)).astype(np.float32)
        nx = (~np.ascontiguousarray(sv).view(np.int32)).view(np.float32)
        yy0 = nx * np.float32(s1)
        yy1 = yy0 * (np.float32(imm2) - sv * yy0)
        return (yy1 * in1).astype(np.float32)

    spec = Spec(body=body, reference=ref)
    opcode = max(dve_ops._SUB_OPCODE_FOR_NAME.values()) + 1
    assert opcode < 0x20
    shas = {}
    for ver in ("v3", "v4"):
        op_spec = DveOpSpec(name=name, opcode=opcode, uops=lower(spec, ver=ver),
                            rd1_en=_has_src1(spec))
        shas[ver] = op_spec.sha(ver)
    op = dve_ops.DveOp(name, spec, subdim=False, uops_sha=shas)
    dve_ops.OPS.append(op)
    dve_ops._SUB_OPCODE_FOR_NAME[name] = opcode
    dve_ops.CUSTOM_DVE_SPECS[name] = spec
    return op


def _install_neff_cache():
    """Cache compiled NEFFs on disk keyed by BIR hash."""
    global _cache_installed
    if _cache_installed:
        return
    _cache_installed = True
    import hashlib
    import shutil

    from concourse import bass2jax, bass_utils

    orig = bass_utils.compile_bir_kernel

    def cached(bir_json, tmpdir, neff_name="file.neff"):
        try:
            os.makedirs(_NEFF_CACHE_DIR, exist_ok=True)
            h = hashlib.sha256(bir_json).hexdigest()[:32]
            cpath = os.path.join(_NEFF_CACHE_DIR, h + ".neff")
            if os.path.exists(cpath):
                dst = os.path.join(tmpdir, neff_name)
                shutil.copy(cpath, dst)
                return dst
        except Exception:
            cpath = None
        out = orig(bir_json, tmpdir, neff_name)
        if cpath is not None:
            try:
                shutil.copy(out, cpath)
            except Exception:
                pass
        return out

    bass_utils.compile_bir_kernel = cached
    bass2jax.compile_bir_kernel = cached


def _expand_maps():
    """E: [atom a, (j,r)] one-hot; G: [row r, (j,a)] one-hot / 12."""
    emat = np.zeros((SA, RT * 128), dtype=np.float32)
    gmat = np.zeros((128, RT * 128), dtype=np.float32)
    for j in range(RT):
        for r in range(128):
            a = (128 * j + r) // M
            emat[a, j * 128 + r] = 1.0
            gmat[r, j * 128 + a] = 1.0 / M
    return emat, gmat


def _build(general_ln1, general_ln2):
    import concourse.bass as bass
    import concourse.tile as tile
    from concourse import bacc, mybir
    from concourse.hw_specs import get_activation_tables

    f32 = mybir.dt.float32
    f32r = mybir.dt.float32r
    bf16 = mybir.dt.bfloat16
    i16 = mybir.dt.int16
    AF = mybir.ActivationFunctionType

    global _SPSIG_OP
    _SPSIG_OP = _register_sp_sigmoid_op()

    nc = bacc.Bacc("TRN2", target_bir_lowering=False, debug=False,
                   num_devices=NCORES, num_swdge_queues=4)

    d_atom_bf16 = nc.dram_tensor("atom_bf16", [NPAD, A], bf16,
                                 kind="ExternalInput")
    d_atomT = nc.dram_tensor("atomT2", [128, 2 * NS], f32r, kind="ExternalInput")
    d_atom_rows = nc.dram_tensor("atom_rows", [NS, A], f32, kind="ExternalInput")
    d_bondT = nc.dram_tensor("bondT", [B, ROWS], bf16, kind="ExternalInput")
    d_idx16 = nc.dram_tensor("idx16", [128, ROWS // 16], i16,
                             kind="ExternalInput")
    d_wt = nc.dram_tensor("wt", [128, NCHUNK * K2A], f32r, kind="ExternalInput")
    d_wtn = nc.dram_tensor("wtn_bf16", [128, 3 * K2A], bf16,
                           kind="ExternalInput")
    d_fcb = nc.dram_tensor("fcb_rep", [128, K2A], f32, kind="ExternalInput")
    d_emat = nc.dram_tensor("emat", [SA, RT * 128], f32r, kind="ExternalInput")
    d_gmat = nc.dram_tensor("gmat", [128, RT * 128], f32r, kind="ExternalInput")
    d_ident = nc.dram_tensor("ident_bf16", [128, 128], bf16,
                             kind="ExternalInput")
    if general_ln1:
        d_g1 = nc.dram_tensor("ln1g_rep", [128, K2A], f32, kind="ExternalInput")
        d_b1 = nc.dram_tensor("ln1b_rep", [128, K2A], f32, kind="ExternalInput")
    if general_ln2:
        d_g2 = nc.dram_tensor("ln2g_rep", [128, A], f32, kind="ExternalInput")
        d_b2 = nc.dram_tensor("ln2b_rep", [128, A], f32, kind="ExternalInput")
    d_out = nc.dram_tensor("out", [NS, A], f32, kind="ExternalOutput")

    r = lambda ap: ap if ap.dtype == f32r else ap.bitcast(f32r)

    # Index of the combined ln+exp activation table set.  The table-load
    # placement pass inserts a load wherever an activation's function is
    # missing from the currently-loaded set, picking the FIRST set that
    # contains it -- Exp alone resolves to the exp-only set and Ln to the
    # ln-only set, so a kernel interleaving them would reload tables twice
    # per row-tile (~1.3us each).  Pre-loading the combined set (which has
    # both) up front satisfies every activation and the pass adds nothing.
    lnexp_set_id = list(get_activation_tables(nc.m.arch).keys()).index(
        "natural_log_exp_and_others")

    with tile.TileContext(nc) as tc:
        nc.scalar.add_instruction(mybir.InstLoadActFuncSet(
            name=nc.get_next_instruction_name(), ins=[], outs=[],
            act_func_set_id=lnexp_set_id))
        with (
            tc.tile_pool(name="const", bufs=1) as cpool,
            tc.tile_pool(name="io", bufs=3) as iopool,
            tc.tile_pool(name="work", bufs=4) as wpool,
            tc.tile_pool(name="stat", bufs=4) as spool,
            tc.tile_pool(name="gat", bufs=2 * RT) as gatpool,
            tc.tile_pool(name="zps", bufs=STAT_BB, space="PSUM") as zpool,
            tc.tile_pool(name="tps", bufs=2, space="PSUM") as tpool,
            tc.tile_pool(name="aps", bufs=1, space="PSUM") as apool,
            tc.tile_pool(name="gps", bufs=2, space="PSUM") as gpool,
        ):
            # ---- resident constants ----
            wt = cpool.tile([128, NCHUNK * K2A], f32r, tag="wt")
            nc.sync.dma_start(wt[:], d_wt[:])
            wtn = cpool.tile([128, 3 * K2A], bf16, tag="wtn")
            nc.sync.dma_start(wtn[:], d_wtn[:])
            atomT = cpool.tile([128, 2 * NS], f32r, tag="atomT")
            nc.sync.dma_start(atomT[:], d_atomT[:])
            fcb = cpool.tile([128, K2A], f32, tag="fcb")
            nc.sync.dma_start(fcb[:], d_fcb[:])
            emat = cpool.tile([SA, RT * 128], f32r, tag="emat")
            nc.sync.dma_start(emat[:], d_emat[:])
            gmat = cpool.tile([128, RT * 128], f32r, tag="gmat")
            nc.sync.dma_start(gmat[:], d_gmat[:])
            ident = cpool.tile([128, 128], bf16, tag="ident")
            nc.sync.dma_start(ident[:], d_ident[:])
            eps_t = cpool.tile([128, 1], f32, tag="eps")
            nc.gpsimd.memset(eps_t[:], LN_EPS)
            ones_t = cpool.tile([128, 1], f32, tag="ones")
            nc.gpsimd.memset(ones_t[:], 1.0)
            if general_ln1:
                g1 = cpool.tile([128, K2A], f32, tag="g1")
                nc.sync.dma_start(g1[:], d_g1[:])
                b1 = cpool.tile([128, K2A], f32, tag="b1")
                nc.sync.dma_start(b1[:], d_b1[:])
            if general_ln2:
                g2 = cpool.tile([128, A], f32, tag="g2")
                nc.sync.dma_start(g2[:], d_g2[:])
                b2 = cpool.tile([128, A], f32, tag="b2")
                nc.sync.dma_start(b2[:], d_b2[:])

            NB = RT // STAT_BB       # stat batches per supertile

            def emit_epilogue(pend):
                """Deferred per-supertile tail: the 12 G (mean-over-m)
                matmuls + LN2 + residual + output DMA.  Emitted one
                supertile late so the PE never stalls waiting for the
                current supertile's elementwise chain to produce gated."""
                ps, pagg, parows = pend
                st6b = spool.tile([128, 6], f32, tag="st6b")
                nc.vector.bn_stats(st6b[:], pagg[:])
                st2b = spool.tile([128, 2], f32, tag="st2b")
                nc.vector.bn_aggr(st2b[:], st6b[:])
                lnv2 = spool.tile([128, 1], f32, tag="lnv2")
                nc.scalar.activation(lnv2[:], st2b[:, 1:2], AF.Ln,
                                     bias=eps_t[:])
                inv2 = spool.tile([128, 1], f32, tag="inv2")
                nc.scalar.activation(inv2[:], lnv2[:], AF.Exp, scale=-0.5)
                nmi2 = spool.tile([128, 1], f32, tag="nmi2")
                nc.vector.tensor_scalar(
                    out=nmi2[:], in0=st2b[:, 0:1], scalar1=inv2[:],
                    scalar2=-1.0, op0=mybir.AluOpType.mult,
                    op1=mybir.AluOpType.mult,
                )
                normed = wpool.tile([SA, A], f32, tag="normed")
                nc.vector.tensor_scalar(
                    out=normed[:], in0=pagg[:], scalar1=inv2[:],
                    scalar2=nmi2[:], op0=mybir.AluOpType.mult,
                    op1=mybir.AluOpType.add,
                )
                if general_ln2:
                    nc.vector.tensor_mul(out=normed[:], in0=normed[:],
                                         in1=g2[:])
                    nc.vector.tensor_add(out=normed[:], in0=normed[:],
                                         in1=b2[:])
                out_sb = wpool.tile([SA, A], f32, tag="out_sb")
                nc.vector.tensor_add(out=out_sb[:], in0=normed[:],
                                     in1=parows[:])
                nc.sync.dma_start(d_out[ps * SA:(ps + 1) * SA, :], out_sb[:])

            for s in range(NSUPER):
                row0 = s * ST_ROWS
                col0 = s * (ST_ROWS // 16)
                # wrapped int16 indices for this supertile: [128, 96]
                idx = iopool.tile([128, ST_ROWS // 16], i16, tag="idx")
                nc.sync.dma_start(
                    idx[:], d_idx16[:, col0:col0 + ST_ROWS // 16])
                # gathered neighbor rows, row-major bf16, two 768-row halves
                # (transpose-mode dma_gather measured 2x slower on HW; rows
                # land [i%128 partition, i//128 block, 256] and get PE-
                # transposed below).  One gather instruction per half costs
                # ~1.3us on Pool vs 12x1.4us for per-row-tile indirect DMA.
                GH = ST_ROWS // 2     # 768
                nbr_g = iopool.tile([128, M * A], bf16, tag="nbr_g")
                for h in range(2):
                    nc.gpsimd.dma_gather(
                        out_ap=nbr_g[:, h * (GH // 128) * A:
                                     (h + 1) * (GH // 128) * A]
                        .rearrange("p (g n) -> p g n", g=GH // 128),
                        in_ap=d_atom_bf16[:],
                        idxs_ap=idx[:, h * (GH // 16):(h + 1) * (GH // 16)],
                        num_idxs=GH,
                        num_idxs_reg=GH,
                        elem_size=A,
                        transpose=False,
                        queue_num=(2 * s + h) % 4,
                    )
                # bond^T slice [128 feat, 1536 rows] bf16
                bondT = iopool.tile([B, ST_ROWS], bf16, tag="bondT")
                nc.sync.dma_start(bondT[:], d_bondT[:, row0:row0 + ST_ROWS])
                # residual rows
                arows = iopool.tile([SA, A], f32, tag="arows")
                nc.sync.dma_start(arows[:], d_atom_rows[s * SA:(s + 1) * SA, :])

                # atom contribution for these 128 atoms: [128 atoms, 512]
                ap_ps = apool.tile([SA, K2A], f32, tag="ap_ps")
                for c in range(2):
                    nc.tensor.matmul(
                        out=ap_ps[:],
                        lhsT=r(atomT[:, c * NS + s * SA: c * NS + (s + 1) * SA]),
                        rhs=r(wt[:, c * K2A:(c + 1) * K2A]),
                        start=(c == 0), stop=(c == 1),
                    )
                ap_sb = wpool.tile([SA, K2A], f32r, tag="ap_sb")
                nc.vector.tensor_add(out=ap_sb[:], in0=ap_ps[:], in1=fcb[:])

                agg = gpool.tile([SA, A], f32, tag="agg")
                gated_tiles = []

                for bb in range(NB):
                    zs = []
                    stb = spool.tile([128, 2 * STAT_BB], f32, tag="stb")
                    for b in range(STAT_BB):
                        j = bb * STAT_BB + b
                        # transpose gathered rows -> [feat, rows] bf16
                        tp = tpool.tile([128, A], bf16, tag="tp")
                        for c in range(2):
                            nc.tensor.transpose(
                                out=tp[:, c * 128:(c + 1) * 128],
                                in_=nbr_g[:, j * A + c * 128:
                                          j * A + (c + 1) * 128],
                                identity=ident[:],
                            )
                        nbrT = wpool.tile([128, A], bf16, tag="nbrT")
                        nc.scalar.copy(nbrT[:], tp[:])

                        # z = E@atom_part + nbrT'@W2 + bondT'@W3  [128 rows, 512]
                        z = zpool.tile([128, K2A], f32, tag="z")
                        nc.tensor.matmul(
                            out=z[:],
                            lhsT=r(emat[:, j * 128:(j + 1) * 128]),
                            rhs=r(ap_sb[:]),
                            start=True, stop=False,
                        )
                        for c in range(2):
                            nc.tensor.matmul(
                                out=z[:],
                                lhsT=nbrT[:, c * 128:(c + 1) * 128],
                                rhs=wtn[:, c * K2A:(c + 1) * K2A],
                                start=False, stop=False,
                            )
                        nc.tensor.matmul(
                            out=z[:],
                            lhsT=bondT[:, j * 128:(j + 1) * 128],
                            rhs=wtn[:, 2 * K2A:3 * K2A],
                            start=False, stop=True,
                        )
                        zs.append(z)
                        # LN1 stats into batch slot b
                        st6 = spool.tile([128, 6], f32, tag="st6")
                        nc.vector.bn_stats(st6[:], z[:])
                        nc.vector.bn_aggr(stb[:, 2 * b:2 * b + 2], st6[:])

                    # batched rsqrt chain over STAT_BB row-tiles
                    mu4 = stb[:, 0:2 * STAT_BB:2]
                    var4 = stb[:, 1:2 * STAT_BB:2]
                    lnv4 = spool.tile([128, STAT_BB], f32, tag="lnv4")
                    nc.scalar.activation(lnv4[:], var4, AF.Ln, bias=eps_t[:])
                    inv4 = spool.tile([128, STAT_BB], f32, tag="inv4")
                    nc.scalar.activation(inv4[:], lnv4[:], AF.Exp, scale=-0.5)
                    ninv4 = spool.tile([128, STAT_BB], f32, tag="ninv4")
                    nc.vector.tensor_scalar(
                        out=ninv4[:], in0=inv4[:], scalar1=-1.0, scalar2=None,
                        op0=mybir.AluOpType.mult,
                    )
                    pnmi4 = spool.tile([128, STAT_BB], f32, tag="pnmi4")
                    nc.vector.tensor_mul(out=pnmi4[:], in0=mu4, in1=inv4[:])
                    nmi4 = spool.tile([128, STAT_BB], f32, tag="nmi4")
                    nc.vector.scalar_tensor_tensor(
                        out=nmi4[:], in0=mu4, scalar=-1.0, in1=inv4[:],
                        op0=mybir.AluOpType.mult, op1=mybir.AluOpType.mult,
                    )

                    for b in range(STAT_BB):
                        j = bb * STAT_BB + b
                        z = zs[b]
                        # gate*core = ln(1+e^v) / (1+e^-u), LN1 fused via
                        # per-partition scale/bias
                        e_u = wpool.tile([128, A], f32, tag="e_u")
                        e_v = wpool.tile([128, A], f32, tag="e_v")
                        if general_ln1:
                            y = wpool.tile([128, K2A], f32, tag="y")
                            nc.vector.tensor_scalar(
                                out=y[:], in0=z[:],
                                scalar1=inv4[:, b:b + 1],
                                scalar2=nmi4[:, b:b + 1],
                                op0=mybir.AluOpType.mult,
                                op1=mybir.AluOpType.add,
                            )
                            nc.vector.tensor_mul(out=y[:], in0=y[:], in1=g1[:])
                            nc.vector.tensor_add(out=y[:], in0=y[:], in1=b1[:])
                            nc.scalar.activation(e_u[:], y[:, :A], AF.Exp,
                                                 scale=-1.0)
                            nc.scalar.activation(e_v[:], y[:, A:], AF.Exp)
                        else:
                            nc.scalar.activation(
                                e_u[:], z[:, :A], AF.Exp,
                                bias=pnmi4[:, b:b + 1], scale=ninv4[:, b:b + 1])
                            nc.scalar.activation(
                                e_v[:], z[:, A:], AF.Exp,
                                bias=nmi4[:, b:b + 1], scale=inv4[:, b:b + 1])
                        sp = wpool.tile([128, A], f32, tag="sp")
                        nc.scalar.activation(sp[:], e_v[:], AF.Ln,
                                             bias=ones_t[:])
                        # NOTE: keep elementwise OFF gpsimd -- mixing Pool
                        # tensor ops with Pool DMA ops forces a ~6us ucode
                        # library reload per switch (HW-measured +1.5ms).
                        # gated = sp/(1+e_u) in ONE custom DVE op.
                        gated = gatpool.tile([128, A], f32r, tag="gated")
                        nc.vector._custom_dve(
                            _SPSIG_OP, out=gated[:],
                            in0=e_u[:], in1=sp[:],
                            s0=_SPSIG_CONSTS[0], s1=_SPSIG_CONSTS[1],
                            imm2=_SPSIG_CONSTS[2],
                        )
                        gated_tiles.append(gated)

                # All 12 G (mean-over-m) matmuls AFTER the full z stream:
                # PE executes its queue in order, so a G matmul emitted
                # mid-stream would stall PE on the DVE/ACT elementwise
                # chain producing gated[j] instead of running the next
                # row-tile's z matmuls.
                for j in range(RT):
                    nc.tensor.matmul(
                        out=agg[:],
                        lhsT=r(gmat[:, j * 128:(j + 1) * 128]),
                        rhs=r(gated_tiles[j][:]),
                        start=(j == 0), stop=(j == RT - 1),
                    )

                emit_epilogue((s, agg, arows))

    nc.compile()
    return nc


def _prep_inputs(atom_feats, bond_feats, fc_w, fc_b, ln1_g, ln1_b, ln2_g,
                 ln2_b, nbr_indices, general_ln1, general_ln2):
    import ml_dtypes

    atom_feats = np.ascontiguousarray(atom_feats, dtype=np.float32)
    pad = NPAD - N
    atom_pad = np.concatenate(
        [atom_feats, np.zeros((pad, A), np.float32)], axis=0)
    bond_pad = np.concatenate(
        [np.asarray(bond_feats, np.float32),
         np.zeros((pad, M, B), np.float32)], axis=0)
    idx_pad = np.concatenate(
        [np.asarray(nbr_indices).astype(np.int32),
         np.zeros((pad, M), np.int32)], axis=0)

    atom_bf16 = atom_pad.astype(ml_dtypes.bfloat16)

    wT = np.ascontiguousarray(np.asarray(fc_w, np.float32).T)      # [640,512]
    wt_host = np.concatenate(
        [wT[c * 128:(c + 1) * 128, :] for c in range(NCHUNK)], axis=1)
    wt_host = np.ascontiguousarray(wt_host)                         # [128,2560]
    wtn_host = np.ascontiguousarray(
        wt_host[:, 2 * K2A:5 * K2A].astype(ml_dtypes.bfloat16))     # [128,1536]
    fcb_rep = np.ascontiguousarray(
        np.broadcast_to(np.asarray(fc_b, np.float32), (128, K2A)))
    emat, gmat = _expand_maps()

    common = {"wt": wt_host, "wtn_bf16": wtn_host, "fcb_rep": fcb_rep,
              "emat": emat, "gmat": gmat, "atom_bf16": atom_bf16,
              "ident_bf16": np.eye(128, dtype=np.float32).astype(
                  ml_dtypes.bfloat16)}
    if general_ln1:
        common["ln1g_rep"] = np.ascontiguousarray(
            np.broadcast_to(np.asarray(ln1_g, np.float32), (128, K2A)))
        common["ln1b_rep"] = np.ascontiguousarray(
            np.broadcast_to(np.asarray(ln1_b, np.float32), (128, K2A)))
    if general_ln2:
        common["ln2g_rep"] = np.ascontiguousarray(
            np.broadcast_to(np.asarray(ln2_g, np.float32), (128, A)))
        common["ln2b_rep"] = np.ascontiguousarray(
            np.broadcast_to(np.asarray(ln2_b, np.float32), (128, A)))

    in_maps = []
    for i in range(NCORES):
        lo, hi = i * NS, (i + 1) * NS
        shard_atoms = atom_pad[lo:hi]                               # [3840,256]
        atomT = np.ascontiguousarray(shard_atoms.T)                 # [256,3840]
        atomT2 = np.ascontiguousarray(
            np.concatenate([atomT[:128], atomT[128:]], axis=1))     # [128,7680]
        bond_flat = bond_pad[lo:hi].reshape(ROWS, B)
        bondT = np.ascontiguousarray(
            bond_flat.T.astype(ml_dtypes.bfloat16))                 # [128,46080]
        # int16 indices, wrapped: logical index i at [i % 16, i // 16],
        # replicated down the partition dim for the 8 Q7 cores.
        flat_idx = idx_pad[lo:hi].reshape(ROWS).astype(np.int16)
        idx16 = np.ascontiguousarray(
            np.tile(flat_idx.reshape(ROWS // 16, 16).T, (8, 1)))    # [128,2880]
        m = dict(common)
        m["atomT2"] = atomT2
        m["atom_rows"] = np.ascontiguousarray(shard_atoms)
        m["bondT"] = bondT
        m["idx16"] = idx16
        in_maps.append(m)
    return in_maps


def _run(nc, in_maps, trace=False):
    from concourse.bass_utils import run_bass_kernel_spmd
    _install_neff_cache()
    res = run_bass_kernel_spmd(nc, in_maps, list(range(NCORES)), trace=trace)
    out = np.concatenate(
        [res.results[i]["out"] for i in range(NCORES)], axis=0)[:N]
    return np.ascontiguousarray(out), res


def measure_exec_ns(nc, in_maps, iters=24):
    """Estimate device exec time by pipelining async dispatches.

    No NTFF profiling is available under this axon client, so time N
    back-to-back executions of the resident executable (inputs device-
    resident, no donation) and difference out the fixed dispatch cost.
    """
    import time

    import jax
    from jax.experimental.shard_map import shard_map
    from jax.sharding import Mesh, NamedSharding, PartitionSpec

    from concourse import bass2jax, mybir
    from concourse.bass2jax import _bass_exec_p, partition_id_tensor

    bass2jax.install_neuronx_cc_hook()
    _install_neff_cache()

    partition_name = (nc.partition_id_tensor.name
                      if nc.partition_id_tensor else None)
    in_names, out_names, out_avals, zero_outs = [], [], [], []
    for alloc in nc.m.functions[0].allocations:
        if not isinstance(alloc, mybir.MemoryLocationSet):
            continue
        name = alloc.memorylocations[0].name
        if alloc.kind == "ExternalInput":
            if name != partition_name:
                in_names.append(name)
        elif alloc.kind == "ExternalOutput":
            shape = tuple(alloc.tensor_shape)
            dtype = mybir.dt.np(alloc.dtype)
            out_names.append(name)
            out_avals.append(jax.core.ShapedArray(shape, dtype))
            zero_outs.append(np.zeros(shape, dtype))
    n_params = len(in_names)
    all_in = list(in_names) + list(out_names)
    if partition_name:
        all_in.append(partition_name)

    def _body(*args):
        operands = list(args)
        if partition_name:
            operands.append(partition_id_tensor())
        outs = _bass_exec_p.bind(
            *operands, out_avals=tuple(out_avals), in_names=tuple(all_in),
            out_names=tuple(out_names), lowering_input_output_aliases=(),
            sim_require_finite=True, sim_require_nnan=True, nc=nc)
        return tuple(outs)

    devices = jax.devices()[:NCORES]
    mesh = Mesh(np.asarray(devices), ("core",))
    nin = n_params + len(zero_outs)
    sharded = jax.jit(
        shard_map(_body, mesh=mesh, in_specs=(PartitionSpec("core"),) * nin,
                  out_specs=(PartitionSpec("core"),) * len(out_names),
                  check_rep=False),
        keep_unused=True)
    sh = NamedSharding(mesh, PartitionSpec("core"))
    concat = [np.concatenate([np.asarray(in_maps[c][nm])
                              for c in range(NCORES)], axis=0)
              for nm in in_names]
    concat += [np.zeros((NCORES * z.shape[0], *z.shape[1:]), z.dtype)
               for z in zero_outs]
    dev_in = [jax.device_put(a, sh) for a in concat]

    jax.block_until_ready(sharded(*dev_in))   # compile + warm

    def run_n(n):
        t0 = time.perf_counter()
        rs = [sharded(*dev_in) for _ in range(n)]
        jax.block_until_ready(rs)
        return time.perf_counter() - t0

    run_n(2)
    t_small = min(run_n(2) for _ in range(3))
    t_big = min(run_n(2 + iters) for _ in range(3))
    est_ns = (t_big - t_small) / iters * 1e9
    return est_ns, t_small, t_big


def kernel(atom_feats, bond_feats, fc_w, fc_b, ln1_g, ln1_b, ln2_g, ln2_b,
           nbr_indices, _trace=False, _return_res=False):
    general_ln1 = not (np.allclose(ln1_g, 1.0) and np.allclose(ln1_b, 0.0))
    general_ln2 = not (np.allclose(ln2_g, 1.0) and np.allclose(ln2_b, 0.0))
    key = (general_ln1, general_ln2)
    if key not in _CACHE:
        _CACHE[key] = _build(general_ln1, general_ln2)
    nc = _CACHE[key]
    in_maps = _prep_inputs(atom_feats, bond_feats, fc_w, fc_b, ln1_g, ln1_b,
                           ln2_g, ln2_b, nbr_indices, general_ln1, general_ln2)
    out, res = _run(nc, in_maps, trace=_trace)
    if _return_res:
        return out, res
    return out



# revision 11
# speedup vs baseline: 1.1634x; 1.0600x over previous
"""CGCNNConv forward on 8 Trainium2 NeuronCores (Bass/Tile).

Math (per atom i, neighbor slot m):
  combined = [atom[i] | atom[nbr[i,m]] | bond[i,m]]          # 640
  z        = combined @ fc_w.T + fc_b                        # 512
  z        = LN(z) * ln1_g + ln1_b
  out[i]   = atom[i] + LN( mean_m sigmoid(z[:256]) * softplus(z[256:]) ) * ln2_g + ln2_b

Sharding: atoms split across 8 cores (padded 30000 -> 30720 = 8*3840).
atom_feats is replicated to every core's HBM (bf16) so the neighbor
gather is a local dma_gather.

Device layout per core (supertile = 128 atoms = 1536 (atom,m) rows = 12
row-tiles of 128 rows):
  - atom contribution computed once per atom (atomT stationary, W1^T
    moving) with fc_b folded in, then expanded to rows with a one-hot
    matmul (E).
  - neighbor rows arrive via TWO 768-row dma_gather ops per supertile
    (row-major bf16; transpose-mode gather measured ~2x slower on HW,
    and >768 rows per gather crashes the SWDGE ucode).  Two gather
    instructions replace twelve per-row-tile indirect DMAs, cutting the
    994ns SWDGE fixed cost per op; the DMA stream itself is descriptor-
    rate-bound at ~11.5ns/row.
  - gathered rows are PE-transposed (bf16, 2x[128,128] per row-tile)
    and copied PSUM->SBUF on ACT (Copy lives in every act table set).
  - bond features arrive pre-transposed from the host.
  - z accumulates in PSUM [128 rows, 512]; LN1 stats via bn_stats; the
    rsqrt(var) chain runs on ACT batched over STAT_BB row-tiles
    ([128,BB] Ln+Exp instead of per-tile [128,1] ops).
  - sigmoid/softplus use exp/ln only, all from the combined
    natural_log_exp activation-table set pre-loaded once up front --
    otherwise the table-load pass flip-flops between the exp-only and
    ln-only sets (2 x ~1.3us reload per row-tile, ~1ms total).
  - all elementwise work stays OFF gpsimd: mixing Pool tensor ops with
    Pool DMA ops forces a ~6us ucode library reload per switch
    (HW-measured +1.5ms).
  - mean over m via a one-hot 1/12 matmul (G) accumulating [128 atoms,256]
    across the 12 row-tiles of a supertile; LN2 + residual epilogue.
Matmuls run in float32r (full-rate at N>=256) except the neighbor
contribution (bf16 gather data, bf16 weight copy).
"""

import os
import sys

import numpy as np

sys.path.insert(0, "/opt/trn_rl_repo")
os.environ.setdefault("NEURON_COMPILE_CACHE_URL", "/root/neff_cache")

N, M, A, B = 30000, 12, 256, 128
NCORES = 8
NS = 3840                    # atoms per core (padded)
NPAD = NS * NCORES           # 30720
SA = 128                     # atoms per supertile
NSUPER = NS // SA            # 30
RT = 12                      # row-tiles per supertile
ST_ROWS = SA * M             # 1536
ROWS = NS * M                # 46080
K2A = 2 * A                  # 512
KIN = 2 * A + B              # 640
NCHUNK = KIN // 128          # 5
LN_EPS = 1e-5
STAT_BB = 2                  # row-tiles per batched LN1-rsqrt group

_CACHE = {}
_NEFF_CACHE_DIR = os.environ.get("NEFF_DISK_CACHE", "/root/neff_cache")
_cache_installed = False

_SPSIG_CONSTS = (1.0, -0.23549792, 2.0017324)


def _register_sp_sigmoid_op():
    """Custom DVE op: out = Src1 * approx(1/(Src0 + 1)).

    Fuses the sigmoid-gate divide chain {denom = 1+e_u; rden = 1/denom;
    gated = sp*rden} (3 DVE instructions, ~760ns/row-tile) into ONE
    custom-DVE instruction.  Same BITWISE_NOT exponent-flip seed as
    RECIPROCAL_APPROX_FAST but on (Src0+1), with a single inline NR pass
    (the +1 stage uses the uop slot the second NR pass would need):
    ~0.17% max rel err on the gate, well inside the 2e-2 gate."""
    import numpy as np

    from concourse import dve_ops
    from concourse.dve_spec import AluOp, Bin, C0, C1, C2, Spec, Src0, Src1
    from concourse.dve_spec import _has_src1, lower
    from concourse.dve_uop import DveOpSpec

    name = "SP_SIGMOID_ANT"
    if name in dve_ops._SUB_OPCODE_FOR_NAME:
        return next(op for op in dve_ops.OPS if op.name == name)

    s = Src0 + C0
    not_s = Bin(AluOp.BITWISE_NOT, s, s)
    y0 = not_s * C1
    y1 = y0 * (C2 - s * y0)
    body = y1 * Src1

    def ref(in0, in1, s0, s1, imm2):
        sv = (in0.astype(np.float32) + np.float32(s0)).astype(np.float32)
        nx = (~np.ascontiguousarray(sv).view(np.int32)).view(np.float32)
        yy0 = nx * np.float32(s1)
        yy1 = yy0 * (np.float32(imm2) - sv * yy0)
        return (yy1 * in1).astype(np.float32)

    spec = Spec(body=body, reference=ref)
    opcode = max(dve_ops._SUB_OPCODE_FOR_NAME.values()) + 1
    assert opcode < 0x20
    shas = {}
    for ver in ("v3", "v4"):
        op_spec = DveOpSpec(name=name, opcode=opcode, uops=lower(spec, ver=ver),
                            rd1_en=_has_src1(spec))
        shas[ver] = op_spec.sha(ver)
    op = dve_ops.DveOp(name, spec, subdim=False, uops_sha=shas)
    dve_ops.OPS.append(op)
    dve_ops._SUB_OPCODE_FOR_NAME[name] = opcode
    dve_ops.CUSTOM_DVE_SPECS[name] = spec
    return op


def _install_neff_cache():
    """Cache compiled NEFFs on disk keyed by BIR hash."""
    global _cache_installed
    if _cache_installed:
        return
    _cache_installed = True
    import hashlib
    import shutil

    from concourse import bass2jax, bass_utils

    orig = bass_utils.compile_bir_kernel

    def cached(bir_json, tmpdir, neff_name="file.neff"):
        try:
            os.makedirs(_NEFF_CACHE_DIR, exist_ok=True)
            h = hashlib.sha256(bir_json).hexdigest()[:32]
            cpath = os.path.join(_NEFF_CACHE_DIR, h + ".neff")
            if os.path.exists(cpath):
                dst = os.path.join(tmpdir, neff_name)
                shutil.copy(cpath, dst)
                return dst
        except Exception:
            cpath = None
        out = orig(bir_json, tmpdir, neff_name)
        if cpath is not None:
            try:
                shutil.copy(out, cpath)
            except Exception:
                pass
        return out

    bass_utils.compile_bir_kernel = cached
    bass2jax.compile_bir_kernel = cached


def _expand_maps():
    """E: [atom a, (j,r)] one-hot; G: [row r, (j,a)] one-hot / 12."""
    emat = np.zeros((SA, RT * 128), dtype=np.float32)
    gmat = np.zeros((128, RT * 128), dtype=np.float32)
    for j in range(RT):
        for r in range(128):
            a = (128 * j + r) // M
            emat[a, j * 128 + r] = 1.0
            gmat[r, j * 128 + a] = 1.0 / M
    return emat, gmat


def _build(general_ln1, general_ln2):
    import concourse.bass as bass
    import concourse.tile as tile
    from concourse import bacc, mybir
    from concourse.hw_specs import get_activation_tables

    f32 = mybir.dt.float32
    f32r = mybir.dt.float32r
    bf16 = mybir.dt.bfloat16
    i16 = mybir.dt.int16
    AF = mybir.ActivationFunctionType

    global _SPSIG_OP
    _SPSIG_OP = _register_sp_sigmoid_op()

    nc = bacc.Bacc("TRN2", target_bir_lowering=False, debug=False,
                   num_devices=NCORES, num_swdge_queues=4)

    d_atom_bf16 = nc.dram_tensor("atom_bf16", [NPAD, A], bf16,
                                 kind="ExternalInput")
    d_atomT = nc.dram_tensor("atomT2", [128, 2 * NS], f32r, kind="ExternalInput")
    d_atom_rows = nc.dram_tensor("atom_rows", [NS, A], f32, kind="ExternalInput")
    d_bondT = nc.dram_tensor("bondT", [B, ROWS], bf16, kind="ExternalInput")
    d_idx16 = nc.dram_tensor("idx16", [128, ROWS // 16], i16,
                             kind="ExternalInput")
    d_wt = nc.dram_tensor("wt", [128, NCHUNK * K2A], f32r, kind="ExternalInput")
    d_wtn = nc.dram_tensor("wtn_bf16", [128, 3 * K2A], bf16,
                           kind="ExternalInput")
    d_fcb = nc.dram_tensor("fcb_rep", [128, K2A], f32, kind="ExternalInput")
    d_emat = nc.dram_tensor("emat", [SA, RT * 128], f32r, kind="ExternalInput")
    d_gmat = nc.dram_tensor("gmat", [128, RT * 128], f32r, kind="ExternalInput")
    d_ident = nc.dram_tensor("ident_bf16", [128, 128], bf16,
                             kind="ExternalInput")
    if general_ln1:
        d_g1 = nc.dram_tensor("ln1g_rep", [128, K2A], f32, kind="ExternalInput")
        d_b1 = nc.dram_tensor("ln1b_rep", [128, K2A], f32, kind="ExternalInput")
    if general_ln2:
        d_g2 = nc.dram_tensor("ln2g_rep", [128, A], f32, kind="ExternalInput")
        d_b2 = nc.dram_tensor("ln2b_rep", [128, A], f32, kind="ExternalInput")
    d_out = nc.dram_tensor("out", [NS, A], f32, kind="ExternalOutput")

    r = lambda ap: ap if ap.dtype == f32r else ap.bitcast(f32r)

    # Index of the combined ln+exp activation table set.  The table-load
    # placement pass inserts a load wherever an activation's function is
    # missing from the currently-loaded set, picking the FIRST set that
    # contains it -- Exp alone resolves to the exp-only set and Ln to the
    # ln-only set, so a kernel interleaving them would reload tables twice
    # per row-tile (~1.3us each).  Pre-loading the combined set (which has
    # both) up front satisfies every activation and the pass adds nothing.
    lnexp_set_id = list(get_activation_tables(nc.m.arch).keys()).index(
        "natural_log_exp_and_others")

    with tile.TileContext(nc) as tc:
        nc.scalar.add_instruction(mybir.InstLoadActFuncSet(
            name=nc.get_next_instruction_name(), ins=[], outs=[],
            act_func_set_id=lnexp_set_id))
        with (
            tc.tile_pool(name="const", bufs=1) as cpool,
            tc.tile_pool(name="io", bufs=3) as iopool,
            tc.tile_pool(name="work", bufs=4) as wpool,
            tc.tile_pool(name="stat", bufs=4) as spool,
            tc.tile_pool(name="gat", bufs=2 * RT) as gatpool,
            tc.tile_pool(name="nbrp", bufs=RT) as nbrpool,
            tc.tile_pool(name="zps", bufs=4, space="PSUM") as zpool,
            tc.tile_pool(name="tps", bufs=2, space="PSUM") as tpool,
            tc.tile_pool(name="aps", bufs=1, space="PSUM") as apool,
            tc.tile_pool(name="gps", bufs=1, space="PSUM") as gpool,
        ):
            # ---- resident constants ----
            wt = cpool.tile([128, NCHUNK * K2A], f32r, tag="wt")
            nc.sync.dma_start(wt[:], d_wt[:])
            wtn = cpool.tile([128, 3 * K2A], bf16, tag="wtn")
            nc.sync.dma_start(wtn[:], d_wtn[:])
            atomT = cpool.tile([128, 2 * NS], f32r, tag="atomT")
            nc.sync.dma_start(atomT[:], d_atomT[:])
            fcb = cpool.tile([128, K2A], f32, tag="fcb")
            nc.sync.dma_start(fcb[:], d_fcb[:])
            emat = cpool.tile([SA, RT * 128], f32r, tag="emat")
            nc.sync.dma_start(emat[:], d_emat[:])
            gmat = cpool.tile([128, RT * 128], f32r, tag="gmat")
            nc.sync.dma_start(gmat[:], d_gmat[:])
            ident = cpool.tile([128, 128], bf16, tag="ident")
            nc.sync.dma_start(ident[:], d_ident[:])
            eps_t = cpool.tile([128, 1], f32, tag="eps")
            nc.gpsimd.memset(eps_t[:], LN_EPS)
            ones_t = cpool.tile([128, 1], f32, tag="ones")
            nc.gpsimd.memset(ones_t[:], 1.0)
            if general_ln1:
                g1 = cpool.tile([128, K2A], f32, tag="g1")
                nc.sync.dma_start(g1[:], d_g1[:])
                b1 = cpool.tile([128, K2A], f32, tag="b1")
                nc.sync.dma_start(b1[:], d_b1[:])
            if general_ln2:
                g2 = cpool.tile([128, A], f32, tag="g2")
                nc.sync.dma_start(g2[:], d_g2[:])
                b2 = cpool.tile([128, A], f32, tag="b2")
                nc.sync.dma_start(b2[:], d_b2[:])

            NB = RT // STAT_BB       # stat batches per supertile

            def emit_epilogue(pend):
                """Deferred per-supertile tail: the 12 G (mean-over-m)
                matmuls + LN2 + residual + output DMA.  Emitted one
                supertile late so the PE never stalls waiting for the
                current supertile's elementwise chain to produce gated."""
                ps, pagg, parows = pend
                st6b = spool.tile([128, 6], f32, tag="st6b")
                nc.vector.bn_stats(st6b[:], pagg[:])
                st2b = spool.tile([128, 2], f32, tag="st2b")
                nc.vector.bn_aggr(st2b[:], st6b[:])
                lnv2 = spool.tile([128, 1], f32, tag="lnv2")
                nc.scalar.activation(lnv2[:], st2b[:, 1:2], AF.Ln,
                                     bias=eps_t[:])
                inv2 = spool.tile([128, 1], f32, tag="inv2")
                nc.scalar.activation(inv2[:], lnv2[:], AF.Exp, scale=-0.5)
                nmi2 = spool.tile([128, 1], f32, tag="nmi2")
                nc.vector.tensor_scalar(
                    out=nmi2[:], in0=st2b[:, 0:1], scalar1=inv2[:],
                    scalar2=-1.0, op0=mybir.AluOpType.mult,
                    op1=mybir.AluOpType.mult,
                )
                normed = wpool.tile([SA, A], f32, tag="normed")
                nc.vector.tensor_scalar(
                    out=normed[:], in0=pagg[:], scalar1=inv2[:],
                    scalar2=nmi2[:], op0=mybir.AluOpType.mult,
                    op1=mybir.AluOpType.add,
                )
                if general_ln2:
                    nc.vector.tensor_mul(out=normed[:], in0=normed[:],
                                         in1=g2[:])
                    nc.vector.tensor_add(out=normed[:], in0=normed[:],
                                         in1=b2[:])
                out_sb = wpool.tile([SA, A], f32, tag="out_sb")
                nc.vector.tensor_add(out=out_sb[:], in0=normed[:],
                                     in1=parows[:])
                nc.sync.dma_start(d_out[ps * SA:(ps + 1) * SA, :], out_sb[:])

            for s in range(NSUPER):
                row0 = s * ST_ROWS
                col0 = s * (ST_ROWS // 16)
                # wrapped int16 indices for this supertile: [128, 96]
                idx = iopool.tile([128, ST_ROWS // 16], i16, tag="idx")
                nc.sync.dma_start(
                    idx[:], d_idx16[:, col0:col0 + ST_ROWS // 16])
                # gathered neighbor rows, row-major bf16, two 768-row halves
                # (transpose-mode dma_gather measured 2x slower on HW; rows
                # land [i%128 partition, i//128 block, 256] and get PE-
                # transposed below).  One gather instruction per half costs
                # ~1.3us on Pool vs 12x1.4us for per-row-tile indirect DMA.
                GH = ST_ROWS // 2     # 768
                nbr_g = iopool.tile([128, M * A], bf16, tag="nbr_g")
                for h in range(2):
                    nc.gpsimd.dma_gather(
                        out_ap=nbr_g[:, h * (GH // 128) * A:
                                     (h + 1) * (GH // 128) * A]
                        .rearrange("p (g n) -> p g n", g=GH // 128),
                        in_ap=d_atom_bf16[:],
                        idxs_ap=idx[:, h * (GH // 16):(h + 1) * (GH // 16)],
                        num_idxs=GH,
                        num_idxs_reg=GH,
                        elem_size=A,
                        transpose=False,
                        queue_num=(2 * s + h) % 4,
                    )
                # bond^T slice [128 feat, 1536 rows] bf16
                bondT = iopool.tile([B, ST_ROWS], bf16, tag="bondT")
                nc.sync.dma_start(bondT[:], d_bondT[:, row0:row0 + ST_ROWS])
                # residual rows
                arows = iopool.tile([SA, A], f32, tag="arows")
                nc.sync.dma_start(arows[:], d_atom_rows[s * SA:(s + 1) * SA, :])

                # atom contribution for these 128 atoms: [128 atoms, 512]
                ap_ps = apool.tile([SA, K2A], f32, tag="ap_ps")
                for c in range(2):
                    nc.tensor.matmul(
                        out=ap_ps[:],
                        lhsT=r(atomT[:, c * NS + s * SA: c * NS + (s + 1) * SA]),
                        rhs=r(wt[:, c * K2A:(c + 1) * K2A]),
                        start=(c == 0), stop=(c == 1),
                    )
                ap_sb = wpool.tile([SA, K2A], f32r, tag="ap_sb")
                nc.vector.tensor_add(out=ap_sb[:], in0=ap_ps[:], in1=fcb[:])

                agg = gpool.tile([SA, A], f32, tag="agg")
                gated_tiles = []

                # Transpose + PSUM->SBUF copy for ALL 12 row-tiles up front:
                # the copies sit at the head of the ACT queue so PE's z
                # matmuls never wait behind the (busy) act/stat chain for
                # their nbrT operand.  nbrpool holds all 12 tiles.
                nbrTs = []
                for j in range(RT):
                    tp = tpool.tile([128, A], bf16, tag="tp")
                    for c in range(2):
                        nc.tensor.transpose(
                            out=tp[:, c * 128:(c + 1) * 128],
                            in_=nbr_g[:, j * A + c * 128:
                                      j * A + (c + 1) * 128],
                            identity=ident[:],
                        )
                    nbrT = nbrpool.tile([128, A], bf16, tag="nbrT")
                    nc.scalar.copy(nbrT[:], tp[:])
                    nbrTs.append(nbrT)

                for bb in range(NB):
                    zs = []
                    stb = spool.tile([128, 2 * STAT_BB], f32, tag="stb")
                    for b in range(STAT_BB):
                        j = bb * STAT_BB + b
                        nbrT = nbrTs[j]

                        # z = E@atom_part + nbrT'@W2 + bondT'@W3  [128 rows, 512]
                        z = zpool.tile([128, K2A], f32, tag="z")
                        nc.tensor.matmul(
                            out=z[:],
                            lhsT=r(emat[:, j * 128:(j + 1) * 128]),
                            rhs=r(ap_sb[:]),
                            start=True, stop=False,
                        )
                        for c in range(2):
                            nc.tensor.matmul(
                                out=z[:],
                                lhsT=nbrT[:, c * 128:(c + 1) * 128],
                                rhs=wtn[:, c * K2A:(c + 1) * K2A],
                                start=False, stop=False,
                            )
                        nc.tensor.matmul(
                            out=z[:],
                            lhsT=bondT[:, j * 128:(j + 1) * 128],
                            rhs=wtn[:, 2 * K2A:3 * K2A],
                            start=False, stop=True,
                        )
                        zs.append(z)
                        # LN1 stats into batch slot b
                        st6 = spool.tile([128, 6], f32, tag="st6")
                        nc.vector.bn_stats(st6[:], z[:])
                        nc.vector.bn_aggr(stb[:, 2 * b:2 * b + 2], st6[:])

                    # batched rsqrt chain over STAT_BB row-tiles
                    mu4 = stb[:, 0:2 * STAT_BB:2]
                    var4 = stb[:, 1:2 * STAT_BB:2]
                    lnv4 = spool.tile([128, STAT_BB], f32, tag="lnv4")
                    nc.scalar.activation(lnv4[:], var4, AF.Ln, bias=eps_t[:])
                    inv4 = spool.tile([128, STAT_BB], f32, tag="inv4")
                    nc.scalar.activation(inv4[:], lnv4[:], AF.Exp, scale=-0.5)
                    ninv4 = spool.tile([128, STAT_BB], f32, tag="ninv4")
                    nc.vector.tensor_scalar(
                        out=ninv4[:], in0=inv4[:], scalar1=-1.0, scalar2=None,
                        op0=mybir.AluOpType.mult,
                    )
                    pnmi4 = spool.tile([128, STAT_BB], f32, tag="pnmi4")
                    nc.vector.tensor_mul(out=pnmi4[:], in0=mu4, in1=inv4[:])
                    nmi4 = spool.tile([128, STAT_BB], f32, tag="nmi4")
                    nc.vector.scalar_tensor_tensor(
                        out=nmi4[:], in0=mu4, scalar=-1.0, in1=inv4[:],
                        op0=mybir.AluOpType.mult, op1=mybir.AluOpType.mult,
                    )

                    for b in range(STAT_BB):
                        j = bb * STAT_BB + b
                        z = zs[b]
                        # gate*core = ln(1+e^v) / (1+e^-u), LN1 fused via
                        # per-partition scale/bias
                        e_u = wpool.tile([128, A], f32, tag="e_u")
                        e_v = wpool.tile([128, A], f32, tag="e_v")
                        if general_ln1:
                            y = wpool.tile([128, K2A], f32, tag="y")
                            nc.vector.tensor_scalar(
                                out=y[:], in0=z[:],
                                scalar1=inv4[:, b:b + 1],
                                scalar2=nmi4[:, b:b + 1],
                                op0=mybir.AluOpType.mult,
                                op1=mybir.AluOpType.add,
                            )
                            nc.vector.tensor_mul(out=y[:], in0=y[:], in1=g1[:])
                            nc.vector.tensor_add(out=y[:], in0=y[:], in1=b1[:])
                            nc.scalar.activation(e_u[:], y[:, :A], AF.Exp,
                                                 scale=-1.0)
                            nc.scalar.activation(e_v[:], y[:, A:], AF.Exp)
                        else:
                            nc.scalar.activation(
                                e_u[:], z[:, :A], AF.Exp,
                                bias=pnmi4[:, b:b + 1], scale=ninv4[:, b:b + 1])
                            nc.scalar.activation(
                                e_v[:], z[:, A:], AF.Exp,
                                bias=nmi4[:, b:b + 1], scale=inv4[:, b:b + 1])
                        sp = wpool.tile([128, A], f32, tag="sp")
                        nc.scalar.activation(sp[:], e_v[:], AF.Ln,
                                             bias=ones_t[:])
                        # NOTE: keep elementwise OFF gpsimd -- mixing Pool
                        # tensor ops with Pool DMA ops forces a ~6us ucode
                        # library reload per switch (HW-measured +1.5ms).
                        # gated = sp/(1+e_u) in ONE custom DVE op.
                        gated = gatpool.tile([128, A], f32r, tag="gated")
                        nc.vector._custom_dve(
                            _SPSIG_OP, out=gated[:],
                            in0=e_u[:], in1=sp[:],
                            s0=_SPSIG_CONSTS[0], s1=_SPSIG_CONSTS[1],
                            imm2=_SPSIG_CONSTS[2],
                        )
                        gated_tiles.append(gated)

                # All 12 G (mean-over-m) matmuls AFTER the full z stream:
                # PE executes its queue in order, so a G matmul emitted
                # mid-stream would stall PE on the DVE/ACT elementwise
                # chain producing gated[j] instead of running the next
                # row-tile's z matmuls.
                for j in range(RT):
                    nc.tensor.matmul(
                        out=agg[:],
                        lhsT=r(gmat[:, j * 128:(j + 1) * 128]),
                        rhs=r(gated_tiles[j][:]),
                        start=(j == 0), stop=(j == RT - 1),
                    )

                emit_epilogue((s, agg, arows))

    nc.compile()
    return nc


def _prep_inputs(atom_feats, bond_feats, fc_w, fc_b, ln1_g, ln1_b, ln2_g,
                 ln2_b, nbr_indices, general_ln1, general_ln2):
    import ml_dtypes

    atom_feats = np.ascontiguousarray(atom_feats, dtype=np.float32)
    pad = NPAD - N
    atom_pad = np.concatenate(
        [atom_feats, np.zeros((pad, A), np.float32)], axis=0)
    bond_pad = np.concatenate(
        [np.asarray(bond_feats, np.float32),
         np.zeros((pad, M, B), np.float32)], axis=0)
    idx_pad = np.concatenate(
        [np.asarray(nbr_indices).astype(np.int32),
         np.zeros((pad, M), np.int32)], axis=0)

    atom_bf16 = atom_pad.astype(ml_dtypes.bfloat16)

    wT = np.ascontiguousarray(np.asarray(fc_w, np.float32).T)      # [640,512]
    wt_host = np.concatenate(
        [wT[c * 128:(c + 1) * 128, :] for c in range(NCHUNK)], axis=1)
    wt_host = np.ascontiguousarray(wt_host)                         # [128,2560]
    wtn_host = np.ascontiguousarray(
        wt_host[:, 2 * K2A:5 * K2A].astype(ml_dtypes.bfloat16))     # [128,1536]
    fcb_rep = np.ascontiguousarray(
        np.broadcast_to(np.asarray(fc_b, np.float32), (128, K2A)))
    emat, gmat = _expand_maps()

    common = {"wt": wt_host, "wtn_bf16": wtn_host, "fcb_rep": fcb_rep,
              "emat": emat, "gmat": gmat, "atom_bf16": atom_bf16,
              "ident_bf16": np.eye(128, dtype=np.float32).astype(
                  ml_dtypes.bfloat16)}
    if general_ln1:
        common["ln1g_rep"] = np.ascontiguousarray(
            np.broadcast_to(np.asarray(ln1_g, np.float32), (128, K2A)))
        common["ln1b_rep"] = np.ascontiguousarray(
            np.broadcast_to(np.asarray(ln1_b, np.float32), (128, K2A)))
    if general_ln2:
        common["ln2g_rep"] = np.ascontiguousarray(
            np.broadcast_to(np.asarray(ln2_g, np.float32), (128, A)))
        common["ln2b_rep"] = np.ascontiguousarray(
            np.broadcast_to(np.asarray(ln2_b, np.float32), (128, A)))

    in_maps = []
    for i in range(NCORES):
        lo, hi = i * NS, (i + 1) * NS
        shard_atoms = atom_pad[lo:hi]                               # [3840,256]
        atomT = np.ascontiguousarray(shard_atoms.T)                 # [256,3840]
        atomT2 = np.ascontiguousarray(
            np.concatenate([atomT[:128], atomT[128:]], axis=1))     # [128,7680]
        bond_flat = bond_pad[lo:hi].reshape(ROWS, B)
        bondT = np.ascontiguousarray(
            bond_flat.T.astype(ml_dtypes.bfloat16))                 # [128,46080]
        # int16 indices, wrapped: logical index i at [i % 16, i // 16],
        # replicated down the partition dim for the 8 Q7 cores.
        flat_idx = idx_pad[lo:hi].reshape(ROWS).astype(np.int16)
        idx16 = np.ascontiguousarray(
            np.tile(flat_idx.reshape(ROWS // 16, 16).T, (8, 1)))    # [128,2880]
        m = dict(common)
        m["atomT2"] = atomT2
        m["atom_rows"] = np.ascontiguousarray(shard_atoms)
        m["bondT"] = bondT
        m["idx16"] = idx16
        in_maps.append(m)
    return in_maps


def _run(nc, in_maps, trace=False):
    from concourse.bass_utils import run_bass_kernel_spmd
    _install_neff_cache()
    res = run_bass_kernel_spmd(nc, in_maps, list(range(NCORES)), trace=trace)
    out = np.concatenate(
        [res.results[i]["out"] for i in range(NCORES)], axis=0)[:N]
    return np.ascontiguousarray(out), res


def measure_exec_ns(nc, in_maps, iters=24):
    """Estimate device exec time by pipelining async dispatches.

    No NTFF profiling is available under this axon client, so time N
    back-to-back executions of the resident executable (inputs device-
    resident, no donation) and difference out the fixed dispatch cost.
    """
    import time

    import jax
    from jax.experimental.shard_map import shard_map
    from jax.sharding import Mesh, NamedSharding, PartitionSpec

    from concourse import bass2jax, mybir
    from concourse.bass2jax import _bass_exec_p, partition_id_tensor

    bass2jax.install_neuronx_cc_hook()
    _install_neff_cache()

    partition_name = (nc.partition_id_tensor.name
                      if nc.partition_id_tensor else None)
    in_names, out_names, out_avals, zero_outs = [], [], [], []
    for alloc in nc.m.functions[0].allocations:
        if not isinstance(alloc, mybir.MemoryLocationSet):
            continue
        name = alloc.memorylocations[0].name
        if alloc.kind == "ExternalInput":
            if name != partition_name:
                in_names.append(name)
        elif alloc.kind == "ExternalOutput":
            shape = tuple(alloc.tensor_shape)
            dtype = mybir.dt.np(alloc.dtype)
            out_names.append(name)
            out_avals.append(jax.core.ShapedArray(shape, dtype))
            zero_outs.append(np.zeros(shape, dtype))
    n_params = len(in_names)
    all_in = list(in_names) + list(out_names)
    if partition_name:
        all_in.append(partition_name)

    def _body(*args):
        operands = list(args)
        if partition_name:
            operands.append(partition_id_tensor())
        outs = _bass_exec_p.bind(
            *operands, out_avals=tuple(out_avals), in_names=tuple(all_in),
            out_names=tuple(out_names), lowering_input_output_aliases=(),
            sim_require_finite=True, sim_require_nnan=True, nc=nc)
        return tuple(outs)

    devices = jax.devices()[:NCORES]
    mesh = Mesh(np.asarray(devices), ("core",))
    nin = n_params + len(zero_outs)
    sharded = jax.jit(
        shard_map(_body, mesh=mesh, in_specs=(PartitionSpec("core"),) * nin,
                  out_specs=(PartitionSpec("core"),) * len(out_names),
                  check_rep=False),
        keep_unused=True)
    sh = NamedSharding(mesh, PartitionSpec("core"))
    concat = [np.concatenate([np.asarray(in_maps[c][nm])
                              for c in range(NCORES)], axis=0)
              for nm in in_names]
    concat += [np.zeros((NCORES * z.shape[0], *z.shape[1:]), z.dtype)
               for z in zero_outs]
    dev_in = [jax.device_put(a, sh) for a in concat]

    jax.block_until_ready(sharded(*dev_in))   # compile + warm

    def run_n(n):
        t0 = time.perf_counter()
        rs = [sharded(*dev_in) for _ in range(n)]
        jax.block_until_ready(rs)
        return time.perf_counter() - t0

    run_n(2)
    t_small = min(run_n(2) for _ in range(3))
    t_big = min(run_n(2 + iters) for _ in range(3))
    est_ns = (t_big - t_small) / iters * 1e9
    return est_ns, t_small, t_big


def kernel(atom_feats, bond_feats, fc_w, fc_b, ln1_g, ln1_b, ln2_g, ln2_b,
           nbr_indices, _trace=False, _return_res=False):
    general_ln1 = not (np.allclose(ln1_g, 1.0) and np.allclose(ln1_b, 0.0))
    general_ln2 = not (np.allclose(ln2_g, 1.0) and np.allclose(ln2_b, 0.0))
    key = (general_ln1, general_ln2)
    if key not in _CACHE:
        _CACHE[key] = _build(general_ln1, general_ln2)
    nc = _CACHE[key]
    in_maps = _prep_inputs(atom_feats, bond_feats, fc_w, fc_b, ln1_g, ln1_b,
                           ln2_g, ln2_b, nbr_indices, general_ln1, general_ln2)
    out, res = _run(nc, in_maps, trace=_trace)
    if _return_res:
        return out, res
    return out



# revision 13
# speedup vs baseline: 1.3055x; 1.1222x over previous
"""CGCNNConv forward on 8 Trainium2 NeuronCores (Bass/Tile).

Math (per atom i, neighbor slot m):
  combined = [atom[i] | atom[nbr[i,m]] | bond[i,m]]          # 640
  z        = combined @ fc_w.T + fc_b                        # 512
  z        = LN(z) * ln1_g + ln1_b
  out[i]   = atom[i] + LN( mean_m sigmoid(z[:256]) * softplus(z[256:]) ) * ln2_g + ln2_b

Sharding: atoms split across 8 cores (padded 30000 -> 30720 = 8*3840).
atom_feats is replicated to every core's HBM (bf16) so the neighbor
gather is a local dma_gather.

Device layout per core (supertile = 128 atoms = 1536 (atom,m) rows = 12
row-tiles of 128 rows):
  - atom contribution computed once per atom (atomT stationary, W1^T
    moving) with fc_b folded in, then expanded to rows with a one-hot
    matmul (E).
  - neighbor rows arrive via TWO 768-row dma_gather ops per supertile
    (row-major bf16; transpose-mode gather measured ~2x slower on HW,
    and >768 rows per gather crashes the SWDGE ucode).  Two gather
    instructions replace twelve per-row-tile indirect DMAs, cutting the
    994ns SWDGE fixed cost per op; the DMA stream itself is descriptor-
    rate-bound at ~11.5ns/row.
  - gathered rows are PE-transposed (bf16, 2x[128,128] per row-tile)
    and copied PSUM->SBUF on ACT (Copy lives in every act table set).
  - bond features arrive pre-transposed from the host.
  - z accumulates in PSUM [128 rows, 512]; LN1 stats via bn_stats; the
    rsqrt(var) chain runs on ACT batched over STAT_BB row-tiles
    ([128,BB] Ln+Exp instead of per-tile [128,1] ops).
  - sigmoid/softplus use exp/ln only, all from the combined
    natural_log_exp activation-table set pre-loaded once up front --
    otherwise the table-load pass flip-flops between the exp-only and
    ln-only sets (2 x ~1.3us reload per row-tile, ~1ms total).
  - all elementwise work stays OFF gpsimd: mixing Pool tensor ops with
    Pool DMA ops forces a ~6us ucode library reload per switch
    (HW-measured +1.5ms).
  - mean over m via a one-hot 1/12 matmul (G) accumulating [128 atoms,256]
    across the 12 row-tiles of a supertile; LN2 + residual epilogue.
Matmuls run in float32r (full-rate at N>=256) except the neighbor
contribution (bf16 gather data, bf16 weight copy).
"""

import os
import sys

import numpy as np

sys.path.insert(0, "/opt/trn_rl_repo")
os.environ.setdefault("NEURON_COMPILE_CACHE_URL", "/root/neff_cache")

N, M, A, B = 30000, 12, 256, 128
NCORES = 8
NS = 3840                    # atoms per core (padded)
NPAD = NS * NCORES           # 30720
SA = 128                     # atoms per supertile
NSUPER = NS // SA            # 30
RT = 12                      # row-tiles per supertile
ST_ROWS = SA * M             # 1536
ROWS = NS * M                # 46080
K2A = 2 * A                  # 512
KIN = 2 * A + B              # 640
NCHUNK = KIN // 128          # 5
LN_EPS = 1e-5
STAT_BB = 2                  # row-tiles per batched LN1-rsqrt group

_CACHE = {}
_NEFF_CACHE_DIR = os.environ.get("NEFF_DISK_CACHE", "/root/neff_cache")
_cache_installed = False

_SPSIG_CONSTS = (1.0, -0.23549792, 2.0017324)


def _register_sp_sigmoid_op():
    """Custom DVE op: out = Src0 * Src1 * approx(1/(Src0 + 1)).

    With Src0 = q = e^{+u}: out = sp * q/(1+q) = sp * sigmoid(u); the
    +u sign lets ONE [128,512] exp activation produce q and e_v with a
    single per-partition scale/bias pair.  Replaces 3 DVE + 1 ACT instruction per row-tile.
    Same BITWISE_NOT exponent-flip seed as RECIPROCAL_APPROX_FAST but on
    (Src0+1), one inline NR pass: ~0.17% max rel err on the gate."""
    import numpy as np

    from concourse import dve_ops
    from concourse.dve_spec import AluOp, Bin, C0, C1, C2, Spec, Src0, Src1
    from concourse.dve_spec import _has_src1, lower
    from concourse.dve_uop import DveOpSpec

    name = "SP_SIGMOID_ANT"
    if name in dve_ops._SUB_OPCODE_FOR_NAME:
        return next(op for op in dve_ops.OPS if op.name == name)

    s = Src0 + C0
    not_s = Bin(AluOp.BITWISE_NOT, s, s)
    y0 = not_s * C1
    y1 = y0 * (C2 - s * y0)
    body = (y1 * Src0) * Src1

    def ref(in0, in1, s0, s1, imm2):
        sv = (in0.astype(np.float32) + np.float32(s0)).astype(np.float32)
        nx = (~np.ascontiguousarray(sv).view(np.int32)).view(np.float32)
        yy0 = nx * np.float32(s1)
        yy1 = yy0 * (np.float32(imm2) - sv * yy0)
        return ((yy1 * in0) * in1).astype(np.float32)

    spec = Spec(body=body, reference=ref)
    opcode = max(dve_ops._SUB_OPCODE_FOR_NAME.values()) + 1
    assert opcode < 0x20
    shas = {}
    for ver in ("v3", "v4"):
        op_spec = DveOpSpec(name=name, opcode=opcode, uops=lower(spec, ver=ver),
                            rd1_en=_has_src1(spec))
        shas[ver] = op_spec.sha(ver)
    op = dve_ops.DveOp(name, spec, subdim=False, uops_sha=shas)
    dve_ops.OPS.append(op)
    dve_ops._SUB_OPCODE_FOR_NAME[name] = opcode
    dve_ops.CUSTOM_DVE_SPECS[name] = spec
    return op


def _install_neff_cache():
    """Cache compiled NEFFs on disk keyed by BIR hash."""
    global _cache_installed
    if _cache_installed:
        return
    _cache_installed = True
    import hashlib
    import shutil

    from concourse import bass2jax, bass_utils

    orig = bass_utils.compile_bir_kernel

    def cached(bir_json, tmpdir, neff_name="file.neff"):
        try:
            os.makedirs(_NEFF_CACHE_DIR, exist_ok=True)
            h = hashlib.sha256(bir_json).hexdigest()[:32]
            cpath = os.path.join(_NEFF_CACHE_DIR, h + ".neff")
            if os.path.exists(cpath):
                dst = os.path.join(tmpdir, neff_name)
                shutil.copy(cpath, dst)
                return dst
        except Exception:
            cpath = None
        out = orig(bir_json, tmpdir, neff_name)
        if cpath is not None:
            try:
                shutil.copy(out, cpath)
            except Exception:
                pass
        return out

    bass_utils.compile_bir_kernel = cached
    bass2jax.compile_bir_kernel = cached


def _expand_maps():
    """E: [atom a, (j,r)] one-hot; G: [row r, (j,a)] one-hot / 12."""
    emat = np.zeros((SA, RT * 128), dtype=np.float32)
    gmat = np.zeros((128, RT * 128), dtype=np.float32)
    for j in range(RT):
        for r in range(128):
            a = (128 * j + r) // M
            emat[a, j * 128 + r] = 1.0
            gmat[r, j * 128 + a] = 1.0 / M
    return emat, gmat


def _build(general_ln1, general_ln2):
    import concourse.bass as bass
    import concourse.tile as tile
    from concourse import bacc, mybir
    from concourse.hw_specs import get_activation_tables

    f32 = mybir.dt.float32
    f32r = mybir.dt.float32r
    bf16 = mybir.dt.bfloat16
    i16 = mybir.dt.int16
    AF = mybir.ActivationFunctionType

    global _SPSIG_OP
    _SPSIG_OP = _register_sp_sigmoid_op()

    nc = bacc.Bacc("TRN2", target_bir_lowering=False, debug=False,
                   num_devices=NCORES, num_swdge_queues=4)

    d_atom_bf16 = nc.dram_tensor("atom_bf16", [NPAD, A], bf16,
                                 kind="ExternalInput")
    d_atomT = nc.dram_tensor("atomT2", [128, 2 * NS], f32r, kind="ExternalInput")
    d_atom_rows = nc.dram_tensor("atom_rows", [NS, A], f32, kind="ExternalInput")
    d_bondT = nc.dram_tensor("bondT", [B, ROWS], bf16, kind="ExternalInput")
    d_idx16 = nc.dram_tensor("idx16", [128, ROWS // 16], i16,
                             kind="ExternalInput")
    d_wt = nc.dram_tensor("wt", [128, NCHUNK * K2A], f32r, kind="ExternalInput")
    d_wtn = nc.dram_tensor("wtn_bf16", [128, 3 * K2A], bf16,
                           kind="ExternalInput")
    d_fcb = nc.dram_tensor("fcb_rep", [128, K2A], f32, kind="ExternalInput")
    d_emat = nc.dram_tensor("emat", [SA, RT * 128], f32r, kind="ExternalInput")
    d_gmat = nc.dram_tensor("gmat", [128, RT * 128], f32r, kind="ExternalInput")
    d_ident = nc.dram_tensor("ident_bf16", [128, 128], bf16,
                             kind="ExternalInput")
    if general_ln1:
        d_g1 = nc.dram_tensor("ln1g_rep", [128, K2A], f32, kind="ExternalInput")
        d_b1 = nc.dram_tensor("ln1b_rep", [128, K2A], f32, kind="ExternalInput")
    if general_ln2:
        d_g2 = nc.dram_tensor("ln2g_rep", [128, A], f32, kind="ExternalInput")
        d_b2 = nc.dram_tensor("ln2b_rep", [128, A], f32, kind="ExternalInput")
    d_out = nc.dram_tensor("out", [NS, A], f32, kind="ExternalOutput")

    r = lambda ap: ap if ap.dtype == f32r else ap.bitcast(f32r)

    # Index of the combined ln+exp activation table set.  The table-load
    # placement pass inserts a load wherever an activation's function is
    # missing from the currently-loaded set, picking the FIRST set that
    # contains it -- Exp alone resolves to the exp-only set and Ln to the
    # ln-only set, so a kernel interleaving them would reload tables twice
    # per row-tile (~1.3us each).  Pre-loading the combined set (which has
    # both) up front satisfies every activation and the pass adds nothing.
    lnexp_set_id = list(get_activation_tables(nc.m.arch).keys()).index(
        "natural_log_exp_and_others")

    with tile.TileContext(nc) as tc:
        nc.scalar.add_instruction(mybir.InstLoadActFuncSet(
            name=nc.get_next_instruction_name(), ins=[], outs=[],
            act_func_set_id=lnexp_set_id))
        with (
            tc.tile_pool(name="const", bufs=1) as cpool,
            tc.tile_pool(name="io", bufs=3) as iopool,
            tc.tile_pool(name="work", bufs=4) as wpool,
            tc.tile_pool(name="stat", bufs=4) as spool,
            tc.tile_pool(name="gat", bufs=2 * RT) as gatpool,
            tc.tile_pool(name="nbrp", bufs=RT) as nbrpool,
            tc.tile_pool(name="zps", bufs=4, space="PSUM") as zpool,
            tc.tile_pool(name="tps", bufs=2, space="PSUM") as tpool,
            tc.tile_pool(name="aps", bufs=1, space="PSUM") as apool,
            tc.tile_pool(name="gps", bufs=1, space="PSUM") as gpool,
        ):
            # ---- resident constants ----
            wt = cpool.tile([128, NCHUNK * K2A], f32r, tag="wt")
            nc.sync.dma_start(wt[:], d_wt[:])
            wtn = cpool.tile([128, 3 * K2A], bf16, tag="wtn")
            nc.sync.dma_start(wtn[:], d_wtn[:])
            atomT = cpool.tile([128, 2 * NS], f32r, tag="atomT")
            nc.sync.dma_start(atomT[:], d_atomT[:])
            fcb = cpool.tile([128, K2A], f32, tag="fcb")
            nc.sync.dma_start(fcb[:], d_fcb[:])
            emat = cpool.tile([SA, RT * 128], f32r, tag="emat")
            nc.sync.dma_start(emat[:], d_emat[:])
            gmat = cpool.tile([128, RT * 128], f32r, tag="gmat")
            nc.sync.dma_start(gmat[:], d_gmat[:])
            ident = cpool.tile([128, 128], bf16, tag="ident")
            nc.sync.dma_start(ident[:], d_ident[:])
            eps_t = cpool.tile([128, 1], f32, tag="eps")
            nc.gpsimd.memset(eps_t[:], LN_EPS)
            ones_t = cpool.tile([128, 1], f32, tag="ones")
            nc.gpsimd.memset(ones_t[:], 1.0)
            if general_ln1:
                g1 = cpool.tile([128, K2A], f32, tag="g1")
                nc.sync.dma_start(g1[:], d_g1[:])
                b1 = cpool.tile([128, K2A], f32, tag="b1")
                nc.sync.dma_start(b1[:], d_b1[:])
            if general_ln2:
                g2 = cpool.tile([128, A], f32, tag="g2")
                nc.sync.dma_start(g2[:], d_g2[:])
                b2 = cpool.tile([128, A], f32, tag="b2")
                nc.sync.dma_start(b2[:], d_b2[:])

            NB = RT // STAT_BB       # stat batches per supertile

            def emit_epilogue(pend):
                """Deferred per-supertile tail: the 12 G (mean-over-m)
                matmuls + LN2 + residual + output DMA.  Emitted one
                supertile late so the PE never stalls waiting for the
                current supertile's elementwise chain to produce gated."""
                ps, pagg, parows = pend
                st6b = spool.tile([128, 6], f32, tag="st6b")
                nc.vector.bn_stats(st6b[:], pagg[:])
                st2b = spool.tile([128, 2], f32, tag="st2b")
                nc.vector.bn_aggr(st2b[:], st6b[:])
                lnv2 = spool.tile([128, 1], f32, tag="lnv2")
                nc.scalar.activation(lnv2[:], st2b[:, 1:2], AF.Ln,
                                     bias=eps_t[:])
                inv2 = spool.tile([128, 1], f32, tag="inv2")
                nc.scalar.activation(inv2[:], lnv2[:], AF.Exp, scale=-0.5)
                nmi2 = spool.tile([128, 1], f32, tag="nmi2")
                nc.vector.tensor_scalar(
                    out=nmi2[:], in0=st2b[:, 0:1], scalar1=inv2[:],
                    scalar2=-1.0, op0=mybir.AluOpType.mult,
                    op1=mybir.AluOpType.mult,
                )
                normed = wpool.tile([SA, A], f32, tag="normed")
                nc.vector.tensor_scalar(
                    out=normed[:], in0=pagg[:], scalar1=inv2[:],
                    scalar2=nmi2[:], op0=mybir.AluOpType.mult,
                    op1=mybir.AluOpType.add,
                )
                if general_ln2:
                    nc.vector.tensor_mul(out=normed[:], in0=normed[:],
                                         in1=g2[:])
                    nc.vector.tensor_add(out=normed[:], in0=normed[:],
                                         in1=b2[:])
                out_sb = wpool.tile([SA, A], f32, tag="out_sb")
                nc.vector.tensor_add(out=out_sb[:], in0=normed[:],
                                     in1=parows[:])
                nc.sync.dma_start(d_out[ps * SA:(ps + 1) * SA, :], out_sb[:])

            for s in range(NSUPER):
                row0 = s * ST_ROWS
                col0 = s * (ST_ROWS // 16)
                # wrapped int16 indices for this supertile: [128, 96]
                idx = iopool.tile([128, ST_ROWS // 16], i16, tag="idx")
                nc.sync.dma_start(
                    idx[:], d_idx16[:, col0:col0 + ST_ROWS // 16])
                # gathered neighbor rows, row-major bf16, two 768-row halves
                # (transpose-mode dma_gather measured 2x slower on HW; rows
                # land [i%128 partition, i//128 block, 256] and get PE-
                # transposed below).  One gather instruction per half costs
                # ~1.3us on Pool vs 12x1.4us for per-row-tile indirect DMA.
                GH = ST_ROWS // 2     # 768
                nbr_g = iopool.tile([128, M * A], bf16, tag="nbr_g")
                for h in range(2):
                    nc.gpsimd.dma_gather(
                        out_ap=nbr_g[:, h * (GH // 128) * A:
                                     (h + 1) * (GH // 128) * A]
                        .rearrange("p (g n) -> p g n", g=GH // 128),
                        in_ap=d_atom_bf16[:],
                        idxs_ap=idx[:, h * (GH // 16):(h + 1) * (GH // 16)],
                        num_idxs=GH,
                        num_idxs_reg=GH,
                        elem_size=A,
                        transpose=False,
                        queue_num=(2 * s + h) % 4,
                    )
                # bond^T slice [128 feat, 1536 rows] bf16
                bondT = iopool.tile([B, ST_ROWS], bf16, tag="bondT")
                nc.sync.dma_start(bondT[:], d_bondT[:, row0:row0 + ST_ROWS])
                # residual rows
                arows = iopool.tile([SA, A], f32, tag="arows")
                nc.sync.dma_start(arows[:], d_atom_rows[s * SA:(s + 1) * SA, :])

                # atom contribution for these 128 atoms: [128 atoms, 512]
                ap_ps = apool.tile([SA, K2A], f32, tag="ap_ps")
                for c in range(2):
                    nc.tensor.matmul(
                        out=ap_ps[:],
                        lhsT=r(atomT[:, c * NS + s * SA: c * NS + (s + 1) * SA]),
                        rhs=r(wt[:, c * K2A:(c + 1) * K2A]),
                        start=(c == 0), stop=(c == 1),
                    )
                ap_sb = wpool.tile([SA, K2A], f32r, tag="ap_sb")
                nc.vector.tensor_add(out=ap_sb[:], in0=ap_ps[:], in1=fcb[:])

                agg = gpool.tile([SA, A], f32, tag="agg")
                gated_tiles = []

                # Transpose + PSUM->SBUF copy for ALL 12 row-tiles up front:
                # the copies sit at the head of the ACT queue so PE's z
                # matmuls never wait behind the (busy) act/stat chain for
                # their nbrT operand.  nbrpool holds all 12 tiles.
                nbrTs = []
                for j in range(RT):
                    tp = tpool.tile([128, A], bf16, tag="tp")
                    for c in range(2):
                        nc.tensor.transpose(
                            out=tp[:, c * 128:(c + 1) * 128],
                            in_=nbr_g[:, j * A + c * 128:
                                      j * A + (c + 1) * 128],
                            identity=ident[:],
                        )
                    nbrT = nbrpool.tile([128, A], bf16, tag="nbrT")
                    nc.scalar.copy(nbrT[:], tp[:])
                    nbrTs.append(nbrT)

                for bb in range(NB):
                    zs = []
                    stb = spool.tile([128, 2 * STAT_BB], f32, tag="stb")
                    for b in range(STAT_BB):
                        j = bb * STAT_BB + b
                        nbrT = nbrTs[j]

                        # z = E@atom_part + nbrT'@W2 + bondT'@W3  [128 rows, 512]
                        z = zpool.tile([128, K2A], f32, tag="z")
                        nc.tensor.matmul(
                            out=z[:],
                            lhsT=r(emat[:, j * 128:(j + 1) * 128]),
                            rhs=r(ap_sb[:]),
                            start=True, stop=False,
                        )
                        for c in range(2):
                            nc.tensor.matmul(
                                out=z[:],
                                lhsT=nbrT[:, c * 128:(c + 1) * 128],
                                rhs=wtn[:, c * K2A:(c + 1) * K2A],
                                start=False, stop=False,
                            )
                        nc.tensor.matmul(
                            out=z[:],
                            lhsT=bondT[:, j * 128:(j + 1) * 128],
                            rhs=wtn[:, 2 * K2A:3 * K2A],
                            start=False, stop=True,
                        )
                        zs.append(z)
                        # LN1 stats into batch slot b
                        st6 = spool.tile([128, 6], f32, tag="st6")
                        nc.vector.bn_stats(st6[:], z[:])
                        nc.vector.bn_aggr(stb[:, 2 * b:2 * b + 2], st6[:])

                    # batched rsqrt chain over STAT_BB row-tiles
                    mu4 = stb[:, 0:2 * STAT_BB:2]
                    var4 = stb[:, 1:2 * STAT_BB:2]
                    lnv4 = spool.tile([128, STAT_BB], f32, tag="lnv4")
                    nc.scalar.activation(lnv4[:], var4, AF.Ln, bias=eps_t[:])
                    inv4 = spool.tile([128, STAT_BB], f32, tag="inv4")
                    nc.scalar.activation(inv4[:], lnv4[:], AF.Exp, scale=-0.5)
                    nmi4 = spool.tile([128, STAT_BB], f32, tag="nmi4")
                    nc.vector.scalar_tensor_tensor(
                        out=nmi4[:], in0=mu4, scalar=-1.0, in1=inv4[:],
                        op0=mybir.AluOpType.mult, op1=mybir.AluOpType.mult,
                    )

                    for b in range(STAT_BB):
                        j = bb * STAT_BB + b
                        z = zs[b]
                        # gate*core = ln(1+e^v) / (1+e^-u), LN1 fused via
                        # per-partition scale/bias
                        e_uv = wpool.tile([128, K2A], f32, tag="e_uv")
                        if general_ln1:
                            y = wpool.tile([128, K2A], f32, tag="y")
                            nc.vector.tensor_scalar(
                                out=y[:], in0=z[:],
                                scalar1=inv4[:, b:b + 1],
                                scalar2=nmi4[:, b:b + 1],
                                op0=mybir.AluOpType.mult,
                                op1=mybir.AluOpType.add,
                            )
                            nc.vector.tensor_mul(out=y[:], in0=y[:], in1=g1[:])
                            nc.vector.tensor_add(out=y[:], in0=y[:], in1=b1[:])
                            nc.scalar.activation(e_uv[:], y[:], AF.Exp)
                        else:
                            # ONE exp over the full row: q=e^{+u} | e^{v},
                            # same per-partition scale/bias for both halves
                            nc.scalar.activation(
                                e_uv[:], z[:], AF.Exp,
                                bias=nmi4[:, b:b + 1], scale=inv4[:, b:b + 1])
                        sp = wpool.tile([128, A], f32, tag="sp")
                        nc.scalar.activation(sp[:], e_uv[:, A:], AF.Ln,
                                             bias=ones_t[:])
                        # NOTE: keep elementwise OFF gpsimd -- mixing Pool
                        # tensor ops with Pool DMA ops forces a ~6us ucode
                        # library reload per switch (HW-measured +1.5ms).
                        # gated = sp/(1+e_u) in ONE custom DVE op.
                        gated = gatpool.tile([128, A], f32r, tag="gated")
                        nc.vector._custom_dve(
                            _SPSIG_OP, out=gated[:],
                            in0=e_uv[:, :A], in1=sp[:],
                            s0=_SPSIG_CONSTS[0], s1=_SPSIG_CONSTS[1],
                            imm2=_SPSIG_CONSTS[2],
                        )
                        gated_tiles.append(gated)

                # All 12 G (mean-over-m) matmuls AFTER the full z stream:
                # PE executes its queue in order, so a G matmul emitted
                # mid-stream would stall PE on the DVE/ACT elementwise
                # chain producing gated[j] instead of running the next
                # row-tile's z matmuls.
                for j in range(RT):
                    nc.tensor.matmul(
                        out=agg[:],
                        lhsT=r(gmat[:, j * 128:(j + 1) * 128]),
                        rhs=r(gated_tiles[j][:]),
                        start=(j == 0), stop=(j == RT - 1),
                    )

                emit_epilogue((s, agg, arows))

    nc.compile()
    return nc


def _prep_inputs(atom_feats, bond_feats, fc_w, fc_b, ln1_g, ln1_b, ln2_g,
                 ln2_b, nbr_indices, general_ln1, general_ln2):
    import ml_dtypes

    atom_feats = np.ascontiguousarray(atom_feats, dtype=np.float32)
    pad = NPAD - N
    atom_pad = np.concatenate(
        [atom_feats, np.zeros((pad, A), np.float32)], axis=0)
    bond_pad = np.concatenate(
        [np.asarray(bond_feats, np.float32),
         np.zeros((pad, M, B), np.float32)], axis=0)
    idx_pad = np.concatenate(
        [np.asarray(nbr_indices).astype(np.int32),
         np.zeros((pad, M), np.int32)], axis=0)

    atom_bf16 = atom_pad.astype(ml_dtypes.bfloat16)

    wT = np.ascontiguousarray(np.asarray(fc_w, np.float32).T)      # [640,512]
    wt_host = np.concatenate(
        [wT[c * 128:(c + 1) * 128, :] for c in range(NCHUNK)], axis=1)
    wt_host = np.ascontiguousarray(wt_host)                         # [128,2560]
    wtn_host = np.ascontiguousarray(
        wt_host[:, 2 * K2A:5 * K2A].astype(ml_dtypes.bfloat16))     # [128,1536]
    fcb_rep = np.ascontiguousarray(
        np.broadcast_to(np.asarray(fc_b, np.float32), (128, K2A)))
    emat, gmat = _expand_maps()

    common = {"wt": wt_host, "wtn_bf16": wtn_host, "fcb_rep": fcb_rep,
              "emat": emat, "gmat": gmat, "atom_bf16": atom_bf16,
              "ident_bf16": np.eye(128, dtype=np.float32).astype(
                  ml_dtypes.bfloat16)}
    if general_ln1:
        common["ln1g_rep"] = np.ascontiguousarray(
            np.broadcast_to(np.asarray(ln1_g, np.float32), (128, K2A)))
        common["ln1b_rep"] = np.ascontiguousarray(
            np.broadcast_to(np.asarray(ln1_b, np.float32), (128, K2A)))
    if general_ln2:
        common["ln2g_rep"] = np.ascontiguousarray(
            np.broadcast_to(np.asarray(ln2_g, np.float32), (128, A)))
        common["ln2b_rep"] = np.ascontiguousarray(
            np.broadcast_to(np.asarray(ln2_b, np.float32), (128, A)))

    in_maps = []
    for i in range(NCORES):
        lo, hi = i * NS, (i + 1) * NS
        shard_atoms = atom_pad[lo:hi]                               # [3840,256]
        atomT = np.ascontiguousarray(shard_atoms.T)                 # [256,3840]
        atomT2 = np.ascontiguousarray(
            np.concatenate([atomT[:128], atomT[128:]], axis=1))     # [128,7680]
        bond_flat = bond_pad[lo:hi].reshape(ROWS, B)
        bondT = np.ascontiguousarray(
            bond_flat.T.astype(ml_dtypes.bfloat16))                 # [128,46080]
        # int16 indices, wrapped: logical index i at [i % 16, i // 16],
        # replicated down the partition dim for the 8 Q7 cores.
        flat_idx = idx_pad[lo:hi].reshape(ROWS).astype(np.int16)
        idx16 = np.ascontiguousarray(
            np.tile(flat_idx.reshape(ROWS // 16, 16).T, (8, 1)))    # [128,2880]
        m = dict(common)
        m["atomT2"] = atomT2
        m["atom_rows"] = np.ascontiguousarray(shard_atoms)
        m["bondT"] = bondT
        m["idx16"] = idx16
        in_maps.append(m)
    return in_maps


def _run(nc, in_maps, trace=False):
    from concourse.bass_utils import run_bass_kernel_spmd
    _install_neff_cache()
    res = run_bass_kernel_spmd(nc, in_maps, list(range(NCORES)), trace=trace)
    out = np.concatenate(
        [res.results[i]["out"] for i in range(NCORES)], axis=0)[:N]
    return np.ascontiguousarray(out), res


def measure_exec_ns(nc, in_maps, iters=24):
    """Estimate device exec time by pipelining async dispatches.

    No NTFF profiling is available under this axon client, so time N
    back-to-back executions of the resident executable (inputs device-
    resident, no donation) and difference out the fixed dispatch cost.
    """
    import time

    import jax
    from jax.experimental.shard_map import shard_map
    from jax.sharding import Mesh, NamedSharding, PartitionSpec

    from concourse import bass2jax, mybir
    from concourse.bass2jax import _bass_exec_p, partition_id_tensor

    bass2jax.install_neuronx_cc_hook()
    _install_neff_cache()

    partition_name = (nc.partition_id_tensor.name
                      if nc.partition_id_tensor else None)
    in_names, out_names, out_avals, zero_outs = [], [], [], []
    for alloc in nc.m.functions[0].allocations:
        if not isinstance(alloc, mybir.MemoryLocationSet):
            continue
        name = alloc.memorylocations[0].name
        if alloc.kind == "ExternalInput":
            if name != partition_name:
                in_names.append(name)
        elif alloc.kind == "ExternalOutput":
            shape = tuple(alloc.tensor_shape)
            dtype = mybir.dt.np(alloc.dtype)
            out_names.append(name)
            out_avals.append(jax.core.ShapedArray(shape, dtype))
            zero_outs.append(np.zeros(shape, dtype))
    n_params = len(in_names)
    all_in = list(in_names) + list(out_names)
    if partition_name:
        all_in.append(partition_name)

    def _body(*args):
        operands = list(args)
        if partition_name:
            operands.append(partition_id_tensor())
        outs = _bass_exec_p.bind(
            *operands, out_avals=tuple(out_avals), in_names=tuple(all_in),
            out_names=tuple(out_names), lowering_input_output_aliases=(),
            sim_require_finite=True, sim_require_nnan=True, nc=nc)
        return tuple(outs)

    devices = jax.devices()[:NCORES]
    mesh = Mesh(np.asarray(devices), ("core",))
    nin = n_params + len(zero_outs)
    sharded = jax.jit(
        shard_map(_body, mesh=mesh, in_specs=(PartitionSpec("core"),) * nin,
                  out_specs=(PartitionSpec("core"),) * len(out_names),
                  check_rep=False),
        keep_unused=True)
    sh = NamedSharding(mesh, PartitionSpec("core"))
    concat = [np.concatenate([np.asarray(in_maps[c][nm])
                              for c in range(NCORES)], axis=0)
              for nm in in_names]
    concat += [np.zeros((NCORES * z.shape[0], *z.shape[1:]), z.dtype)
               for z in zero_outs]
    dev_in = [jax.device_put(a, sh) for a in concat]

    jax.block_until_ready(sharded(*dev_in))   # compile + warm

    def run_n(n):
        t0 = time.perf_counter()
        rs = [sharded(*dev_in) for _ in range(n)]
        jax.block_until_ready(rs)
        return time.perf_counter() - t0

    run_n(2)
    t_small = min(run_n(2) for _ in range(3))
    t_big = min(run_n(2 + iters) for _ in range(3))
    est_ns = (t_big - t_small) / iters * 1e9
    return est_ns, t_small, t_big


def kernel(atom_feats, bond_feats, fc_w, fc_b, ln1_g, ln1_b, ln2_g, ln2_b,
           nbr_indices, _trace=False, _return_res=False):
    general_ln1 = not (np.allclose(ln1_g, 1.0) and np.allclose(ln1_b, 0.0))
    general_ln2 = not (np.allclose(ln2_g, 1.0) and np.allclose(ln2_b, 0.0))
    key = (general_ln1, general_ln2)
    if key not in _CACHE:
        _CACHE[key] = _build(general_ln1, general_ln2)
    nc = _CACHE[key]
    in_maps = _prep_inputs(atom_feats, bond_feats, fc_w, fc_b, ln1_g, ln1_b,
                           ln2_g, ln2_b, nbr_indices, general_ln1, general_ln2)
    out, res = _run(nc, in_maps, trace=_trace)
    if _return_res:
        return out, res
    return out

